# revision 4
# baseline (speedup 1.0000x reference)
"""Multi-head attention (B=2, S=2048, D=1024, H=16, causal) on 8 TRN2 cores.

Sharding: data-parallel over batch x tensor-parallel over heads (Megatron).
Core c handles batch b=c//4 and heads [4g, 4g+4) with g=c%4. Each core
computes its 4 heads' Q/K/V projections, causal attention, and its partial
output projection y_partial = attn_x @ W_o[:, cols].T; the host sums the 4
partials per batch.

Everything on-chip runs in transposed (feature x seq) layout so no
transposes are needed anywhere:
  QT/KT [256, 2048] = W @ x^T,  V [s, 4*65] with a fused ones-column,
  S^T[k, q] = KT_h.T @ QT_h,    P^T = exp(S^T/8) (ACT, scale folded),
  O^T_aug [65, q] = V_aug.T @ P^T  (row 64 = softmax denominator),
  attn^T = O^T[0:64] * bcast(1/denom),  y^T = WoT.T @ attn^T.
"""

import numpy as np
import ml_dtypes

B, S, D, H = 2, 2048, 1024, 16
DK = D // H  # 64
NCORES = 8
GROUPS = 4  # cores per batch
HPC = H // GROUPS  # heads per core = 4
HD = HPC * DK  # head dims per core = 256

BF16 = ml_dtypes.bfloat16

QCHUNK = 512  # q columns processed per softmax block
NCHUNKS = S // QCHUNK  # 4
KTILE = 128  # keys per matmul tile
NKT = S // KTILE  # 16
KSUPER = 2  # k-tiles per exp batch ([128, 1024] activations)

_prog_cache = {}


# --------------------------------------------------------------------------
# walrus workaround: this compiler build allows at most 1 semaphore wait per
# instruction; move excess waits onto NoOps inserted before the instruction.
def _split_excess_waits(nc):
    import concourse.mybir as mybir

    ctr = 0
    for f in nc.m.functions:
        for bb in f.blocks:
            out = []
            changed = False
            for inst in bb.instructions:
                si = inst.sync_info
                if si is not None and si.on_wait and len(si.on_wait) > 1:
                    waits = list(si.on_wait)
                    excess, keep = waits[:-1], waits[-1:]
                    for w in excess:
                        nop = mybir.InstNoOp(name=f"waitsplit-{ctr}", ins=[], outs=[])
                        ctr += 1
                        nop.engine = inst.engine
                        nop.sync_info = mybir.SyncInfo(on_wait=[w], on_update=[])
                        out.append(nop)
                    si.on_wait = keep
                    changed = True
                out.append(inst)
            if changed:
                bb.instructions = out
    return ctr


def _build_program(causal: bool):
    import concourse.bass as bass
    import concourse.mybir as mybir
    import concourse.tile as tile

    fp32 = mybir.dt.float32
    bf16 = mybir.dt.bfloat16

    nc = bass.Bass()

    xqT = nc.dram_tensor("xqT", [D, S], bf16, kind="ExternalInput")
    xkT = nc.dram_tensor("xkT", [D, S], bf16, kind="ExternalInput")
    xvT = nc.dram_tensor("xvT", [D, S], bf16, kind="ExternalInput")
    wqT = nc.dram_tensor("wqT", [D, HD], bf16, kind="ExternalInput")
    wkT = nc.dram_tensor("wkT", [D, HD], bf16, kind="ExternalInput")
    wvT = nc.dram_tensor("wvT", [D, HD], bf16, kind="ExternalInput")
    woT = nc.dram_tensor("woT", [HD, D], bf16, kind="ExternalInput")
    yT = nc.dram_tensor("yT", [D, S], mybir.dt.float32, kind="ExternalOutput")
    maskT = None
    if not causal:
        maskT = nc.dram_tensor("maskT", [S, S], bf16, kind="ExternalInput")

    DT = D // 128  # 8 contraction tiles for the input projections

    with tile.TileContext(nc) as tc:
        with (
            tc.tile_pool(name="wpool", bufs=1) as wpool,
            tc.tile_pool(name="res", bufs=1) as res,
            tc.tile_pool(name="xin", bufs=10) as xin,
            tc.tile_pool(name="small", bufs=1) as small,
        ):
            # ---- weights (resident) ----
            wq_t = [wpool.tile([128, HD], bf16, tag=f"wq{k}", name=f"wq{k}") for k in range(DT)]
            wk_t = [wpool.tile([128, HD], bf16, tag=f"wk{k}", name=f"wk{k}") for k in range(DT)]
            wv_t = [wpool.tile([128, HD], bf16, tag=f"wv{k}", name=f"wv{k}") for k in range(DT)]
            wo_t = [wpool.tile([128, D], bf16, tag=f"wo{k}", name=f"wo{k}") for k in range(HD // 128)]
            for k in range(DT):
                nc.sync.dma_start(out=wq_t[k], in_=wqT[128 * k : 128 * k + 128, :])
                nc.sync.dma_start(out=wk_t[k], in_=wkT[128 * k : 128 * k + 128, :])
                nc.sync.dma_start(out=wv_t[k], in_=wvT[128 * k : 128 * k + 128, :])
            for k in range(HD // 128):
                nc.sync.dma_start(out=wo_t[k], in_=woT[128 * k : 128 * k + 128, :])

            # ---- resident activations ----
            # QT/KT: [128, m, S] with m in {0,1}: rows 128m..128m+127 of [256, S]
            qt = res.tile([128, 2, S], bf16, tag="qt")
            kt = res.tile([128, 2, S], bf16, tag="kt")
            # V (+ ones col): [128, ktile j, head h, 65]
            v_sb = res.tile([128, NKT, HPC, DK + 1], bf16, tag="v")
            # attention output (normalized), transposed: [128, m, S]
            at = res.tile([128, 2, S], bf16, tag="at")

            nc.vector.memset(v_sb[:, :, :, DK : DK + 1], 1.0)

            # ones row for the K=1 broadcast matmul
            ones = small.tile([1, DK], bf16, tag="ones")
            nc.vector.memset(ones, 1.0)

            # causal edge masks: m[x, y] = 1 if y >= x + off else 0
            emasks = {}
            if causal:
                for off in (0, 128, 256, 384):
                    m = small.tile([128, QCHUNK], bf16, tag=f"emask{off}")
                    nc.gpsimd.memset(m, 1.0)
                    nc.gpsimd.affine_select(
                        out=m,
                        in_=m,
                        compare_op=mybir.AluOpType.is_ge,
                        fill=0.0,
                        base=-off,
                        pattern=[[1, QCHUNK]],
                        channel_multiplier=-1,
                    )
                    emasks[off] = m

            # ---- phase 1: projections ----
            with tc.tile_pool(name="pps", bufs=3, space="PSUM") as pps:
                # QT = wqT.T @ xqT   (and KT likewise)
                for name, xdram, w_t, dst in (
                    ("q", xqT, wq_t, qt),
                    ("k", xkT, wk_t, kt),
                ):
                    x_t = []
                    for k in range(DT):
                        xt = xin.tile([128, S], bf16)
                        nc.sync.dma_start(
                            out=xt, in_=xdram[128 * k : 128 * k + 128, :]
                        )
                        x_t.append(xt)
                    for n in range(NCHUNKS):
                        for m in range(2):
                            ps = pps.tile([128, QCHUNK], fp32, tag="proj")
                            for k in range(DT):
                                nc.tensor.matmul(
                                    ps[:, :],
                                    lhsT=w_t[k][:, 128 * m : 128 * m + 128],
                                    rhs=x_t[k][:, QCHUNK * n : QCHUNK * (n + 1)],
                                    start=(k == 0),
                                    stop=(k == DT - 1),
                                )
                            nc.vector.tensor_copy(
                                dst[:, m, QCHUNK * n : QCHUNK * (n + 1)], ps[:, :]
                            )
                # V = xvT.T @ wvT  -> [s, HD], stored per (ktile, head) with ones col
                x_t = []
                for k in range(DT):
                    xt = xin.tile([128, S], bf16)
                    nc.sync.dma_start(out=xt, in_=xvT[128 * k : 128 * k + 128, :])
                    x_t.append(xt)
                for j in range(NKT):
                    ps = pps.tile([128, HD], fp32, tag="vproj")
                    for k in range(DT):
                        nc.tensor.matmul(
                            ps[:, :],
                            lhsT=x_t[k][:, 128 * j : 128 * j + 128],
                            rhs=wv_t[k][:, :],
                            start=(k == 0),
                            stop=(k == DT - 1),
                        )
                    nc.vector.tensor_copy(
                        v_sb[:, j, :, 0:DK],
                        ps.rearrange("p (h d) -> p h d", h=HPC),
                    )

            # ---- phase 2: attention ----
            with (
                tc.tile_pool(name="scps", bufs=2, space="PSUM") as scps,
                tc.tile_pool(name="pvps", bufs=2, space="PSUM") as pvps,
                tc.tile_pool(name="bcps", bufs=2, space="PSUM") as bcps,
                tc.tile_pool(name="pt", bufs=4) as ptp,
                tc.tile_pool(name="srec", bufs=2) as srec,
                tc.tile_pool(name="mload", bufs=4) as mload,
            ):
                for c in range(NCHUNKS):
                    qs = slice(QCHUNK * c, QCHUNK * (c + 1))
                    # number of k-tiles this chunk attends to
                    nkt_c = 4 * (c + 1) if causal else NKT
                    nsup = (nkt_c + KSUPER - 1) // KSUPER
                    for h in range(HPC):
                        mh = h // 2
                        ph = 64 * (h % 2)
                        pv = pvps.tile([DK + 1, QCHUNK], fp32, tag="pv")
                        for s_i in range(nsup):
                            jt = [
                                KSUPER * s_i + j2
                                for j2 in range(KSUPER)
                                if KSUPER * s_i + j2 < nkt_c
                            ]
                            sc = scps.tile([128, KSUPER * QCHUNK], fp32, tag="sc")
                            for j2, j in enumerate(jt):
                                nc.tensor.matmul(
                                    sc[:, QCHUNK * j2 : QCHUNK * (j2 + 1)],
                                    lhsT=kt[ph : ph + DK, mh, 128 * j : 128 * j + 128],
                                    rhs=qt[ph : ph + DK, mh, qs],
                                    start=True,
                                    stop=True,
                                )
                            pt = ptp.tile([128, KSUPER * QCHUNK], bf16, tag="pt")
                            nsc = QCHUNK * len(jt)
                            nc.scalar.activation(
                                out=pt[:, 0:nsc],
                                in_=sc[:, 0:nsc],
                                func=mybir.ActivationFunctionType.Exp,
                                scale=1.0 / np.sqrt(np.float32(DK)),
                            )
                            for j2, j in enumerate(jt):
                                pslice = pt[:, QCHUNK * j2 : QCHUNK * (j2 + 1)]
                                if causal:
                                    off = 128 * j - QCHUNK * c
                                    if off >= 0:
                                        nc.vector.tensor_mul(
                                            out=pslice, in0=pslice, in1=emasks[off]
                                        )
                                else:
                                    mt = mload.tile([128, QCHUNK], bf16, tag="mt")
                                    nc.sync.dma_start(
                                        out=mt,
                                        in_=maskT[128 * j : 128 * j + 128, qs],
                                    )
                                    nc.vector.tensor_mul(
                                        out=pslice, in0=pslice, in1=mt
                                    )
                                nc.tensor.matmul(
                                    pv[:, :],
                                    lhsT=v_sb[:, j, h, :],
                                    rhs=pslice,
                                    start=(j == 0),
                                    stop=(j == nkt_c - 1),
                                )
                        # normalize: attnT = pv[0:DK] * bcast(1/pv[DK])
                        rec = srec.tile([1, QCHUNK], bf16, tag="rec")
                        if not causal:
                            nc.scalar.add(
                                out=pv[DK : DK + 1, :],
                                in_=pv[DK : DK + 1, :],
                                add=1e-30,
                            )
                        with nc.allow_low_precision(reason="softmax denom in bf16"):
                            nc.vector.reciprocal(
                                out=rec[0:1, :], in_=pv[DK : DK + 1, :]
                            )
                        bc = bcps.tile([DK, QCHUNK], fp32, tag="bc")
                        nc.tensor.matmul(
                            bc[:, :],
                            lhsT=ones[0:1, :],
                            rhs=rec[0:1, :],
                            start=True,
                            stop=True,
                        )
                        bcs = srec.tile([DK, QCHUNK], bf16, tag="bcs")
                        nc.vector.tensor_copy(bcs[:, :], bc[:, :])
                        nc.vector.tensor_mul(
                            out=at[ph : ph + DK, mh, qs],
                            in0=pv[0:DK, :],
                            in1=bcs[:, :],
                        )

            # ---- phase 3: output projection  yT = woT.T @ attnT ----
            with (
                tc.tile_pool(name="ops", bufs=4, space="PSUM") as ops,
                tc.tile_pool(name="ostg", bufs=4) as ostg,
            ):
                for mo in range(D // 128):
                    for n in range(NCHUNKS):
                        ps = ops.tile([128, QCHUNK], fp32, tag="out")
                        for k2 in range(HD // 128):
                            nc.tensor.matmul(
                                ps[:, :],
                                lhsT=wo_t[k2][:, 128 * mo : 128 * mo + 128],
                                rhs=at[:, k2, QCHUNK * n : QCHUNK * (n + 1)],
                                start=(k2 == 0),
                                stop=(k2 == HD // 128 - 1),
                            )
                        stg = ostg.tile([128, QCHUNK], fp32, tag="stg")
                        nc.vector.tensor_copy(stg[:, :], ps[:, :])
                        nc.sync.dma_start(
                            out=yT[
                                128 * mo : 128 * mo + 128,
                                QCHUNK * n : QCHUNK * (n + 1),
                            ],
                            in_=stg[:, :],
                        )

    _split_excess_waits(nc)
    return nc


def kernel(query, key, value, mask, W_q, W_k, W_v, W_o):
    from concourse.bass_utils import run_bass_kernel_spmd

    query = np.asarray(query)
    key = np.asarray(key)
    value = np.asarray(value)
    mask = np.asarray(mask)
    W_q = np.asarray(W_q)
    W_k = np.asarray(W_k)
    W_v = np.asarray(W_v)
    W_o = np.asarray(W_o)

    m2 = mask.reshape(mask.shape[-2], mask.shape[-1])
    causal = bool(
        np.array_equal(m2 != 0, np.tril(np.ones((S, S), dtype=bool)))
    )

    if causal not in _prog_cache:
        _prog_cache[causal] = _build_program(causal)
    nc = _prog_cache[causal]

    # host-side shard prep (bf16, transposed)
    xT = {}
    for b in range(B):
        xT[("q", b)] = np.ascontiguousarray(query[b].T).astype(BF16)
        xT[("k", b)] = np.ascontiguousarray(key[b].T).astype(BF16)
        xT[("v", b)] = np.ascontiguousarray(value[b].T).astype(BF16)
    maskT_np = None
    if not causal:
        maskT_np = np.ascontiguousarray((m2 != 0).T).astype(BF16)

    in_maps = []
    for c in range(NCORES):
        b, g = divmod(c, GROUPS)
        rows = slice(HD * g, HD * (g + 1))
        im = {
            "xqT": xT[("q", b)],
            "xkT": xT[("k", b)],
            "xvT": xT[("v", b)],
            "wqT": np.ascontiguousarray(W_q[rows, :].T).astype(BF16),
            "wkT": np.ascontiguousarray(W_k[rows, :].T).astype(BF16),
            "wvT": np.ascontiguousarray(W_v[rows, :].T).astype(BF16),
            "woT": np.ascontiguousarray(W_o[:, rows].T).astype(BF16),
        }
        if not causal:
            im["maskT"] = maskT_np
        in_maps.append(im)

    res = run_bass_kernel_spmd(nc, in_maps, core_ids=list(range(NCORES)))

    out = np.zeros((B, S, D), dtype=np.float32)
    for c in range(NCORES):
        b = c // GROUPS
        out[b] += res.results[c]["yT"].T
    return out


# revision 10
# speedup vs baseline: 1.2081x; 1.2081x over previous
"""Multi-head attention (B=2, S=2048, D=1024, H=16, causal) on 8 TRN2 cores.

Sharding: data-parallel over batch x tensor-parallel over heads (Megatron).
Core c handles batch b=c//4 and heads [4g, 4g+4) with g=c%4. Each core
computes its 4 heads' Q/K/V projections, causal attention, and its partial
output projection y_partial = attn_x @ W_o[:, cols].T; the host sums the 4
partials per batch.

Everything on-chip runs in transposed (feature x seq) layout so no
transposes are needed anywhere:
  QT/KT [256, 2048] = W @ x^T,  V [s, 4*65] with a fused ones-column,
  S^T[k, q] = KT_h.T @ QT_h,    P^T = exp(S^T/8) (ACT, scale folded),
  O^T_aug [65, q] = V_aug.T @ P^T  (row 64 = softmax denominator),
  attn^T = O^T[0:64] * bcast(1/denom),  y^T = WoT.T @ attn^T.
"""

import numpy as np
import ml_dtypes

B, S, D, H = 2, 2048, 1024, 16
DK = D // H  # 64
NCORES = 8
GROUPS = 4  # cores per batch
HPC = H // GROUPS  # heads per core = 4
HD = HPC * DK  # head dims per core = 256

BF16 = ml_dtypes.bfloat16

QCHUNK = 512  # q columns processed per softmax block
NCHUNKS = S // QCHUNK  # 4
KTILE = 128  # keys per matmul tile
NKT = S // KTILE  # 16
KSUPER = 2  # k-tiles per exp batch ([128, 1024] activations)

_prog_cache = {}


# --------------------------------------------------------------------------
# walrus workaround: this compiler build allows at most 1 semaphore wait per
# instruction; move excess waits onto NoOps inserted before the instruction.
def _split_excess_waits(nc):
    import concourse.mybir as mybir

    ctr = 0
    for f in nc.m.functions:
        for bb in f.blocks:
            out = []
            changed = False
            for inst in bb.instructions:
                si = inst.sync_info
                if si is not None and si.on_wait and len(si.on_wait) > 1:
                    waits = list(si.on_wait)
                    excess, keep = waits[:-1], waits[-1:]
                    for w in excess:
                        nop = mybir.InstNoOp(name=f"waitsplit-{ctr}", ins=[], outs=[])
                        ctr += 1
                        nop.engine = inst.engine
                        nop.sync_info = mybir.SyncInfo(on_wait=[w], on_update=[])
                        out.append(nop)
                    si.on_wait = keep
                    changed = True
                out.append(inst)
            if changed:
                bb.instructions = out
    return ctr


def _build_program(causal: bool):
    import concourse.bass as bass
    import concourse.mybir as mybir
    import concourse.tile as tile

    fp32 = mybir.dt.float32
    bf16 = mybir.dt.bfloat16

    nc = bass.Bass()

    xqT = nc.dram_tensor("xqT", [D, S], bf16, kind="ExternalInput")
    xkT = nc.dram_tensor("xkT", [D, S], bf16, kind="ExternalInput")
    xvT = nc.dram_tensor("xvT", [D, S], bf16, kind="ExternalInput")
    wqT = nc.dram_tensor("wqT", [D, HD], bf16, kind="ExternalInput")
    wkT = nc.dram_tensor("wkT", [D, HD], bf16, kind="ExternalInput")
    wvT = nc.dram_tensor("wvT", [D, HD], bf16, kind="ExternalInput")
    woT = nc.dram_tensor("woT", [HD, D], bf16, kind="ExternalInput")
    yT = nc.dram_tensor("yT", [D, S], mybir.dt.float32, kind="ExternalOutput")
    maskT = None
    if not causal:
        maskT = nc.dram_tensor("maskT", [S, S], bf16, kind="ExternalInput")

    DT = D // 128  # 8 contraction tiles for the input projections

    with tile.TileContext(nc) as tc:
        with (
            tc.tile_pool(name="wpool", bufs=1) as wpool,
            tc.tile_pool(name="res", bufs=1) as res,
            tc.tile_pool(name="xin", bufs=10) as xin,
            tc.tile_pool(name="small", bufs=1) as small,
        ):
            # ---- weights (resident) ----
            wq_t = [wpool.tile([128, HD], bf16, tag=f"wq{k}", name=f"wq{k}") for k in range(DT)]
            wk_t = [wpool.tile([128, HD], bf16, tag=f"wk{k}", name=f"wk{k}") for k in range(DT)]
            wv_t = [wpool.tile([128, HD], bf16, tag=f"wv{k}", name=f"wv{k}") for k in range(DT)]
            wo_t = [wpool.tile([128, D], bf16, tag=f"wo{k}", name=f"wo{k}") for k in range(HD // 128)]
            for k in range(DT):
                nc.sync.dma_start(out=wq_t[k], in_=wqT[128 * k : 128 * k + 128, :])
                nc.sync.dma_start(out=wk_t[k], in_=wkT[128 * k : 128 * k + 128, :])
                nc.sync.dma_start(out=wv_t[k], in_=wvT[128 * k : 128 * k + 128, :])
            for k in range(HD // 128):
                nc.sync.dma_start(out=wo_t[k], in_=woT[128 * k : 128 * k + 128, :])

            # ---- resident activations ----
            # QT/KT: [128, m, S] with m in {0,1}: rows 128m..128m+127 of [256, S]
            qt = res.tile([128, 2, S], bf16, tag="qt")
            kt = res.tile([128, 2, S], bf16, tag="kt")
            # V (+ ones col): [128, ktile j, head h, 65]
            v_sb = res.tile([128, NKT, HPC, DK + 1], bf16, tag="v")
            # attention output (normalized), transposed: [128, m, S]
            at = res.tile([128, 2, S], bf16, tag="at")

            nc.vector.memset(v_sb[:, :, :, DK : DK + 1], 1.0)

            # ones row for the K=1 broadcast matmul
            ones = small.tile([128, DK], bf16, tag="ones")
            nc.vector.memset(ones, 1.0)

            # causal edge masks: m[x, y] = 1 if y >= x + off else 0
            emasks = {}
            if causal:
                for off in (0, 128, 256, 384):
                    m = small.tile([128, QCHUNK], bf16, tag=f"emask{off}")
                    nc.gpsimd.memset(m, 1.0)
                    nc.gpsimd.affine_select(
                        out=m,
                        in_=m,
                        compare_op=mybir.AluOpType.is_ge,
                        fill=0.0,
                        base=-off,
                        pattern=[[1, QCHUNK]],
                        channel_multiplier=-1,
                    )
                    emasks[off] = m

            # ---- phase 1: projections ----
            with tc.tile_pool(name="pps", bufs=3, space="PSUM") as pps:
                # QT = wqT.T @ xqT   (and KT likewise)
                for name, xdram, w_t, dst in (
                    ("q", xqT, wq_t, qt),
                    ("k", xkT, wk_t, kt),
                ):
                    x_t = []
                    for k in range(DT):
                        xt = xin.tile([128, S], bf16)
                        nc.sync.dma_start(
                            out=xt, in_=xdram[128 * k : 128 * k + 128, :]
                        )
                        x_t.append(xt)
                    for n in range(NCHUNKS):
                        for m in range(2):
                            ps = pps.tile([128, QCHUNK], fp32, tag="proj")
                            for k in range(DT):
                                nc.tensor.matmul(
                                    ps[:, :],
                                    lhsT=w_t[k][:, 128 * m : 128 * m + 128],
                                    rhs=x_t[k][:, QCHUNK * n : QCHUNK * (n + 1)],
                                    start=(k == 0),
                                    stop=(k == DT - 1),
                                )
                            nc.scalar.copy(
                                out=dst[:, m, QCHUNK * n : QCHUNK * (n + 1)],
                                in_=ps[:, :],
                            )
                # V = xvT.T @ wvT  -> [s, HD], stored per (ktile, head) with ones col
                x_t = []
                for k in range(DT):
                    xt = xin.tile([128, S], bf16)
                    nc.sync.dma_start(out=xt, in_=xvT[128 * k : 128 * k + 128, :])
                    x_t.append(xt)
                for j in range(NKT):
                    ps = pps.tile([128, HD], fp32, tag="vproj")
                    for k in range(DT):
                        nc.tensor.matmul(
                            ps[:, :],
                            lhsT=x_t[k][:, 128 * j : 128 * j + 128],
                            rhs=wv_t[k][:, :],
                            start=(k == 0),
                            stop=(k == DT - 1),
                        )
                    nc.scalar.copy(
                        out=v_sb[:, j, :, 0:DK],
                        in_=ps.rearrange("p (h d) -> p h d", h=HPC),
                    )

            # ---- phase 2: attention ----
            with (
                tc.tile_pool(name="scps", bufs=2, space="PSUM") as scps,
                tc.tile_pool(name="pvps", bufs=4, space="PSUM") as pvps,
                tc.tile_pool(name="pt", bufs=6) as ptp,
                tc.tile_pool(name="srec", bufs=2) as srec,
                tc.tile_pool(name="mload", bufs=4) as mload,
            ):
                for c in range(NCHUNKS):
                    qs = slice(QCHUNK * c, QCHUNK * (c + 1))
                    # number of k-tiles this chunk attends to
                    nkt_c = 4 * (c + 1) if causal else NKT
                    nsup = (nkt_c + KSUPER - 1) // KSUPER
                    pvs = []
                    for h in range(HPC):
                        mh = h // 2
                        ph = 64 * (h % 2)
                        pv = pvps.tile([128, QCHUNK], fp32, tag="pv")
                        pvs.append(pv)
                        for s_i in range(nsup):
                            jt = [
                                KSUPER * s_i + j2
                                for j2 in range(KSUPER)
                                if KSUPER * s_i + j2 < nkt_c
                            ]
                            sc = scps.tile([128, KSUPER * QCHUNK], fp32, tag="sc")
                            for j2, j in enumerate(jt):
                                nc.tensor.matmul(
                                    sc[:, QCHUNK * j2 : QCHUNK * (j2 + 1)],
                                    lhsT=kt[ph : ph + DK, mh, 128 * j : 128 * j + 128],
                                    rhs=qt[ph : ph + DK, mh, qs],
                                    start=True,
                                    stop=True,
                                )
                            pt = ptp.tile([128, KSUPER * QCHUNK], bf16, tag="pt")
                            nsc = QCHUNK * len(jt)
                            nc.scalar.activation(
                                out=pt[:, 0:nsc],
                                in_=sc[:, 0:nsc],
                                func=mybir.ActivationFunctionType.Exp,
                                scale=1.0 / np.sqrt(np.float32(DK)),
                            )
                            for j2, j in enumerate(jt):
                                pslice = pt[:, QCHUNK * j2 : QCHUNK * (j2 + 1)]
                                if causal:
                                    off = 128 * j - QCHUNK * c
                                    if off >= 0:
                                        nc.vector.tensor_mul(
                                            out=pslice, in0=pslice, in1=emasks[off]
                                        )
                                else:
                                    mt = mload.tile([128, QCHUNK], bf16, tag="mt")
                                    nc.sync.dma_start(
                                        out=mt,
                                        in_=maskT[128 * j : 128 * j + 128, qs],
                                    )
                                    nc.vector.tensor_mul(
                                        out=pslice, in0=pslice, in1=mt
                                    )
                                nc.tensor.matmul(
                                    pv[0 : DK + 1, :],
                                    lhsT=v_sb[:, j, h, :],
                                    rhs=pslice,
                                    start=(j == 0),
                                    stop=(j == nkt_c - 1),
                                )
                    # normalize all 4 heads: one strided reciprocal per chunk,
                    # bcast 1/denom into rows 64..127 of each pv bank via a
                    # K=1 matmul, then attnT = pv[0:64] * pv[64:128].
                    recf = srec.tile([128, QCHUNK], fp32, tag="recf")
                    recb = srec.tile([128, QCHUNK], bf16, tag="recb")
                    for h in range(HPC):
                        if not causal:
                            nc.scalar.add(
                                out=pvs[h][DK : DK + 1, :],
                                in_=pvs[h][DK : DK + 1, :],
                                add=1e-30,
                            )
                        nc.vector.tensor_copy(
                            recf[32 * h : 32 * h + 1, :],
                            pvs[h][DK : DK + 1, :],
                        )
                    with nc.allow_low_precision(reason="softmax denom in bf16"):
                        nc.vector.reciprocal(
                            out=recb[0:97, :], in_=recf[0:97, :]
                        )
                    for h in range(HPC):
                        mh = h // 2
                        ph = 64 * (h % 2)
                        nc.tensor.matmul(
                            pvs[h][64:128, :],
                            lhsT=ones[32 * h : 32 * h + 1, :],
                            rhs=recb[32 * h : 32 * h + 1, :],
                            start=True,
                            stop=True,
                            tile_position=(32 * h, 64),
                        )
                        bcs = srec.tile([DK, QCHUNK], bf16, tag="bcs")
                        nc.vector.tensor_copy(bcs[:, :], pvs[h][64:128, :])
                        nc.vector.tensor_mul(
                            out=at[ph : ph + DK, mh, qs],
                            in0=pvs[h][0:DK, :],
                            in1=bcs[:, :],
                        )

            # ---- phase 3: output projection  yT = woT.T @ attnT ----
            with (
                tc.tile_pool(name="ops", bufs=4, space="PSUM") as ops,
                tc.tile_pool(name="ostg", bufs=4) as ostg,
            ):
                for mo in range(D // 128):
                    for n in range(NCHUNKS):
                        ps = ops.tile([128, QCHUNK], fp32, tag="out")
                        for k2 in range(HD // 128):
                            nc.tensor.matmul(
                                ps[:, :],
                                lhsT=wo_t[k2][:, 128 * mo : 128 * mo + 128],
                                rhs=at[:, k2, QCHUNK * n : QCHUNK * (n + 1)],
                                start=(k2 == 0),
                                stop=(k2 == HD // 128 - 1),
                            )
                        stg = ostg.tile([128, QCHUNK], fp32, tag="stg")
                        nc.scalar.copy(out=stg[:, :], in_=ps[:, :])
                        nc.sync.dma_start(
                            out=yT[
                                128 * mo : 128 * mo + 128,
                                QCHUNK * n : QCHUNK * (n + 1),
                            ],
                            in_=stg[:, :],
                        )

    _split_excess_waits(nc)
    return nc


def kernel(query, key, value, mask, W_q, W_k, W_v, W_o):
    from concourse.bass_utils import run_bass_kernel_spmd

    query = np.asarray(query)
    key = np.asarray(key)
    value = np.asarray(value)
    mask = np.asarray(mask)
    W_q = np.asarray(W_q)
    W_k = np.asarray(W_k)
    W_v = np.asarray(W_v)
    W_o = np.asarray(W_o)

    m2 = mask.reshape(mask.shape[-2], mask.shape[-1])
    causal = bool(
        np.array_equal(m2 != 0, np.tril(np.ones((S, S), dtype=bool)))
    )

    if causal not in _prog_cache:
        _prog_cache[causal] = _build_program(causal)
    nc = _prog_cache[causal]

    # host-side shard prep (bf16, transposed)
    xT = {}
    for b in range(B):
        xT[("q", b)] = np.ascontiguousarray(query[b].T).astype(BF16)
        xT[("k", b)] = np.ascontiguousarray(key[b].T).astype(BF16)
        xT[("v", b)] = np.ascontiguousarray(value[b].T).astype(BF16)
    maskT_np = None
    if not causal:
        maskT_np = np.ascontiguousarray((m2 != 0).T).astype(BF16)

    in_maps = []
    for c in range(NCORES):
        b, g = divmod(c, GROUPS)
        rows = slice(HD * g, HD * (g + 1))
        im = {
            "xqT": xT[("q", b)],
            "xkT": xT[("k", b)],
            "xvT": xT[("v", b)],
            "wqT": np.ascontiguousarray(W_q[rows, :].T).astype(BF16),
            "wkT": np.ascontiguousarray(W_k[rows, :].T).astype(BF16),
            "wvT": np.ascontiguousarray(W_v[rows, :].T).astype(BF16),
            "woT": np.ascontiguousarray(W_o[:, rows].T).astype(BF16),
        }
        if not causal:
            im["maskT"] = maskT_np
        in_maps.append(im)

    res = run_bass_kernel_spmd(nc, in_maps, core_ids=list(range(NCORES)))

    out = np.zeros((B, S, D), dtype=np.float32)
    for c in range(NCORES):
        b = c // GROUPS
        out[b] += res.results[c]["yT"].T
    return out


# revision 12
# speedup vs baseline: 1.2325x; 1.0202x over previous
"""Multi-head attention (B=2, S=2048, D=1024, H=16, causal) on 8 TRN2 cores.

Sharding: data-parallel over batch x tensor-parallel over heads (Megatron).
Core c handles batch b=c//4 and heads [4g, 4g+4) with g=c%4. Each core
computes its 4 heads' Q/K/V projections, causal attention, and its partial
output projection y_partial = attn_x @ W_o[:, cols].T; the host sums the 4
partials per batch.

Everything on-chip runs in transposed (feature x seq) layout so no
transposes are needed anywhere:
  QT/KT [256, 2048] = W @ x^T,  V [s, 4*65] with a fused ones-column,
  S^T[k, q] = KT_h.T @ QT_h,    P^T = exp(S^T/8) (ACT, scale folded),
  O^T_aug [65, q] = V_aug.T @ P^T  (row 64 = softmax denominator),
  attn^T = O^T[0:64] * bcast(1/denom),  y^T = WoT.T @ attn^T.
"""

import numpy as np
import ml_dtypes

B, S, D, H = 2, 2048, 1024, 16
DK = D // H  # 64
NCORES = 8
GROUPS = 4  # cores per batch
HPC = H // GROUPS  # heads per core = 4
HD = HPC * DK  # head dims per core = 256

BF16 = ml_dtypes.bfloat16

QCHUNK = 512  # q columns processed per softmax block
NCHUNKS = S // QCHUNK  # 4
KTILE = 128  # keys per matmul tile
NKT = S // KTILE  # 16
KSUPER = 2  # k-tiles per exp batch ([128, 1024] activations)

_prog_cache = {}


# --------------------------------------------------------------------------
# walrus workaround: this compiler build allows at most 1 semaphore wait per
# instruction; move excess waits onto NoOps inserted before the instruction.
def _split_excess_waits(nc):
    import concourse.mybir as mybir

    ctr = 0
    for f in nc.m.functions:
        for bb in f.blocks:
            out = []
            changed = False
            for inst in bb.instructions:
                si = inst.sync_info
                if si is not None and si.on_wait and len(si.on_wait) > 1:
                    waits = list(si.on_wait)
                    excess, keep = waits[:-1], waits[-1:]
                    for w in excess:
                        nop = mybir.InstNoOp(name=f"waitsplit-{ctr}", ins=[], outs=[])
                        ctr += 1
                        nop.engine = inst.engine
                        nop.sync_info = mybir.SyncInfo(on_wait=[w], on_update=[])
                        out.append(nop)
                    si.on_wait = keep
                    changed = True
                out.append(inst)
            if changed:
                bb.instructions = out
    return ctr


def _build_program(causal: bool):
    import concourse.bass as bass
    import concourse.mybir as mybir
    import concourse.tile as tile

    fp32 = mybir.dt.float32
    bf16 = mybir.dt.bfloat16

    nc = bass.Bass()

    xqT = nc.dram_tensor("xqT", [D, S], bf16, kind="ExternalInput")
    xkT = nc.dram_tensor("xkT", [D, S], bf16, kind="ExternalInput")
    xvT = nc.dram_tensor("xvT", [D, S], bf16, kind="ExternalInput")
    wqT = nc.dram_tensor("wqT", [D, HD], bf16, kind="ExternalInput")
    wkT = nc.dram_tensor("wkT", [D, HD], bf16, kind="ExternalInput")
    wvT = nc.dram_tensor("wvT", [D, HD], bf16, kind="ExternalInput")
    woT = nc.dram_tensor("woT", [HD, D], bf16, kind="ExternalInput")
    yT = nc.dram_tensor("yT", [D, S], mybir.dt.float32, kind="ExternalOutput")
    maskT = None
    if not causal:
        maskT = nc.dram_tensor("maskT", [S, S], bf16, kind="ExternalInput")

    DT = D // 128  # 8 contraction tiles for the input projections

    with tile.TileContext(nc) as tc:
        with (
            tc.tile_pool(name="wpool", bufs=1) as wpool,
            tc.tile_pool(name="res", bufs=1) as res,
            tc.tile_pool(name="xin", bufs=10) as xin,
            tc.tile_pool(name="small", bufs=1) as small,
        ):
            # ---- weights (resident) ----
            wq_t = [wpool.tile([128, HD], bf16, tag=f"wq{k}", name=f"wq{k}") for k in range(DT)]
            wk_t = [wpool.tile([128, HD], bf16, tag=f"wk{k}", name=f"wk{k}") for k in range(DT)]
            wv_t = [wpool.tile([128, HD], bf16, tag=f"wv{k}", name=f"wv{k}") for k in range(DT)]
            wo_t = [wpool.tile([128, D], bf16, tag=f"wo{k}", name=f"wo{k}") for k in range(HD // 128)]
            for k in range(DT):
                nc.sync.dma_start(out=wq_t[k], in_=wqT[128 * k : 128 * k + 128, :])
                nc.sync.dma_start(out=wk_t[k], in_=wkT[128 * k : 128 * k + 128, :])
                nc.sync.dma_start(out=wv_t[k], in_=wvT[128 * k : 128 * k + 128, :])
            for k in range(HD // 128):
                nc.sync.dma_start(out=wo_t[k], in_=woT[128 * k : 128 * k + 128, :])

            # ---- resident activations ----
            # QT/KT: [128, m, S] with m in {0,1}: rows 128m..128m+127 of [256, S]
            qt = res.tile([128, 2, S], bf16, tag="qt")
            kt = res.tile([128, 2, S], bf16, tag="kt")
            # V (+ ones col): [128, ktile j, head h, 65]
            v_sb = res.tile([128, NKT, HPC, DK + 1], bf16, tag="v")
            # attention output (normalized), transposed: [128, m, S]
            at = res.tile([128, 2, S], bf16, tag="at")

            nc.vector.memset(v_sb[:, :, :, DK : DK + 1], 1.0)

            # ones row for the K=1 broadcast matmul
            ones = small.tile([128, DK], bf16, tag="ones")
            nc.vector.memset(ones, 1.0)

            # causal edge masks: m[x, y] = 1 if y >= x + off else 0
            emasks = {}
            if causal:
                for off in (0, 128, 256, 384):
                    m = small.tile([128, QCHUNK], bf16, tag=f"emask{off}")
                    nc.gpsimd.memset(m, 1.0)
                    nc.gpsimd.affine_select(
                        out=m,
                        in_=m,
                        compare_op=mybir.AluOpType.is_ge,
                        fill=0.0,
                        base=-off,
                        pattern=[[1, QCHUNK]],
                        channel_multiplier=-1,
                    )
                    emasks[off] = m

            # ---- phase 1: projections ----
            with tc.tile_pool(name="pps", bufs=3, space="PSUM") as pps:
                # QT = wqT.T @ xqT   (and KT likewise)
                for name, xdram, w_t, dst in (
                    ("q", xqT, wq_t, qt),
                    ("k", xkT, wk_t, kt),
                ):
                    x_t = []
                    for k in range(DT):
                        xt = xin.tile([128, S], bf16)
                        nc.sync.dma_start(
                            out=xt, in_=xdram[128 * k : 128 * k + 128, :]
                        )
                        x_t.append(xt)
                    for n in range(NCHUNKS):
                        for m in range(2):
                            ps = pps.tile([128, QCHUNK], fp32, tag="proj")
                            for k in range(DT):
                                nc.tensor.matmul(
                                    ps[:, :],
                                    lhsT=w_t[k][:, 128 * m : 128 * m + 128],
                                    rhs=x_t[k][:, QCHUNK * n : QCHUNK * (n + 1)],
                                    start=(k == 0),
                                    stop=(k == DT - 1),
                                )
                            nc.scalar.copy(
                                out=dst[:, m, QCHUNK * n : QCHUNK * (n + 1)],
                                in_=ps[:, :],
                            )
                # V = xvT.T @ wvT  -> [s, HD], stored per (ktile, head) with ones col
                x_t = []
                for k in range(DT):
                    xt = xin.tile([128, S], bf16)
                    nc.sync.dma_start(out=xt, in_=xvT[128 * k : 128 * k + 128, :])
                    x_t.append(xt)
                for j in range(NKT):
                    ps = pps.tile([128, HD], fp32, tag="vproj")
                    for k in range(DT):
                        nc.tensor.matmul(
                            ps[:, :],
                            lhsT=x_t[k][:, 128 * j : 128 * j + 128],
                            rhs=wv_t[k][:, :],
                            start=(k == 0),
                            stop=(k == DT - 1),
                        )
                    nc.scalar.copy(
                        out=v_sb[:, j, :, 0:DK],
                        in_=ps.rearrange("p (h d) -> p h d", h=HPC),
                    )

            # ---- phase 2: attention ----
            with (
                tc.tile_pool(name="scps", bufs=2, space="PSUM") as scps,
                tc.tile_pool(name="pvps", bufs=4, space="PSUM") as pvps,
                tc.tile_pool(name="pt", bufs=6) as ptp,
                tc.tile_pool(name="srec", bufs=2) as srec,
                tc.tile_pool(name="mload", bufs=4) as mload,
            ):
                for c in range(NCHUNKS):
                    qs = slice(QCHUNK * c, QCHUNK * (c + 1))
                    # number of k-tiles this chunk attends to
                    nkt_c = 4 * (c + 1) if causal else NKT
                    nsup = (nkt_c + KSUPER - 1) // KSUPER
                    pvs = [
                        pvps.tile([128, QCHUNK], fp32, tag="pv", name=f"pv{c}_{h}")
                        for h in range(HPC)
                    ]
                    # interleave the 4 heads per k-super: 4 independent
                    # QK -> exp -> mask -> PV chains keep PE busy while ACT
                    # works through the exps.
                    for s_i in range(nsup):
                        jt = [
                            KSUPER * s_i + j2
                            for j2 in range(KSUPER)
                            if KSUPER * s_i + j2 < nkt_c
                        ]
                        for h in range(HPC):
                            mh = h // 2
                            ph = 64 * (h % 2)
                            pv = pvs[h]
                            sc = scps.tile(
                                [128, KSUPER * QCHUNK], fp32, tag="sc",
                                name=f"sc{c}_{s_i}_{h}",
                            )
                            for j2, j in enumerate(jt):
                                nc.tensor.matmul(
                                    sc[:, QCHUNK * j2 : QCHUNK * (j2 + 1)],
                                    lhsT=kt[ph : ph + DK, mh, 128 * j : 128 * j + 128],
                                    rhs=qt[ph : ph + DK, mh, qs],
                                    start=True,
                                    stop=True,
                                )
                            pt = ptp.tile(
                                [128, KSUPER * QCHUNK], bf16, tag="pt",
                                name=f"pt{c}_{s_i}_{h}",
                            )
                            nsc = QCHUNK * len(jt)
                            nc.scalar.activation(
                                out=pt[:, 0:nsc],
                                in_=sc[:, 0:nsc],
                                func=mybir.ActivationFunctionType.Exp,
                                scale=1.0 / np.sqrt(np.float32(DK)),
                            )
                            for j2, j in enumerate(jt):
                                pslice = pt[:, QCHUNK * j2 : QCHUNK * (j2 + 1)]
                                if causal:
                                    off = 128 * j - QCHUNK * c
                                    if off >= 0:
                                        nc.vector.tensor_mul(
                                            out=pslice, in0=pslice, in1=emasks[off]
                                        )
                                else:
                                    mt = mload.tile([128, QCHUNK], bf16, tag="mt")
                                    nc.sync.dma_start(
                                        out=mt,
                                        in_=maskT[128 * j : 128 * j + 128, qs],
                                    )
                                    nc.vector.tensor_mul(
                                        out=pslice, in0=pslice, in1=mt
                                    )
                                nc.tensor.matmul(
                                    pv[0 : DK + 1, :],
                                    lhsT=v_sb[:, j, h, :],
                                    rhs=pslice,
                                    start=(j == 0),
                                    stop=(j == nkt_c - 1),
                                )
                    # normalize all 4 heads: one strided reciprocal per chunk,
                    # bcast 1/denom into rows 64..127 of each pv bank via a
                    # K=1 matmul, then attnT = pv[0:64] * pv[64:128].
                    recf = srec.tile([128, QCHUNK], fp32, tag="recf")
                    recb = srec.tile([128, QCHUNK], bf16, tag="recb")
                    for h in range(HPC):
                        if not causal:
                            nc.scalar.add(
                                out=pvs[h][DK : DK + 1, :],
                                in_=pvs[h][DK : DK + 1, :],
                                add=1e-30,
                            )
                        nc.vector.tensor_copy(
                            recf[32 * h : 32 * h + 1, :],
                            pvs[h][DK : DK + 1, :],
                        )
                    with nc.allow_low_precision(reason="softmax denom in bf16"):
                        nc.vector.reciprocal(
                            out=recb[0:97, :], in_=recf[0:97, :]
                        )
                    for h in range(HPC):
                        mh = h // 2
                        ph = 64 * (h % 2)
                        nc.tensor.matmul(
                            pvs[h][64:128, :],
                            lhsT=ones[32 * h : 32 * h + 1, :],
                            rhs=recb[32 * h : 32 * h + 1, :],
                            start=True,
                            stop=True,
                            tile_position=(32 * h, 64),
                        )
                        bcs = srec.tile([DK, QCHUNK], bf16, tag="bcs")
                        nc.vector.tensor_copy(bcs[:, :], pvs[h][64:128, :])
                        nc.vector.tensor_mul(
                            out=at[ph : ph + DK, mh, qs],
                            in0=pvs[h][0:DK, :],
                            in1=bcs[:, :],
                        )

            # ---- phase 3: output projection  yT = woT.T @ attnT ----
            with (
                tc.tile_pool(name="ops", bufs=4, space="PSUM") as ops,
                tc.tile_pool(name="ostg", bufs=4) as ostg,
            ):
                for mo in range(D // 128):
                    for n in range(NCHUNKS):
                        ps = ops.tile([128, QCHUNK], fp32, tag="out")
                        for k2 in range(HD // 128):
                            nc.tensor.matmul(
                                ps[:, :],
                                lhsT=wo_t[k2][:, 128 * mo : 128 * mo + 128],
                                rhs=at[:, k2, QCHUNK * n : QCHUNK * (n + 1)],
                                start=(k2 == 0),
                                stop=(k2 == HD // 128 - 1),
                            )
                        stg = ostg.tile([128, QCHUNK], fp32, tag="stg")
                        if (mo + n) % 2 == 0:
                            nc.scalar.copy(out=stg[:, :], in_=ps[:, :])
                        else:
                            nc.vector.tensor_copy(stg[:, :], ps[:, :])
                        nc.sync.dma_start(
                            out=yT[
                                128 * mo : 128 * mo + 128,
                                QCHUNK * n : QCHUNK * (n + 1),
                            ],
                            in_=stg[:, :],
                        )

    _split_excess_waits(nc)
    return nc


def kernel(query, key, value, mask, W_q, W_k, W_v, W_o):
    from concourse.bass_utils import run_bass_kernel_spmd

    query = np.asarray(query)
    key = np.asarray(key)
    value = np.asarray(value)
    mask = np.asarray(mask)
    W_q = np.asarray(W_q)
    W_k = np.asarray(W_k)
    W_v = np.asarray(W_v)
    W_o = np.asarray(W_o)

    m2 = mask.reshape(mask.shape[-2], mask.shape[-1])
    causal = bool(
        np.array_equal(m2 != 0, np.tril(np.ones((S, S), dtype=bool)))
    )

    if causal not in _prog_cache:
        _prog_cache[causal] = _build_program(causal)
    nc = _prog_cache[causal]

    # host-side shard prep (bf16, transposed)
    xT = {}
    for b in range(B):
        xT[("q", b)] = np.ascontiguousarray(query[b].T).astype(BF16)
        xT[("k", b)] = np.ascontiguousarray(key[b].T).astype(BF16)
        xT[("v", b)] = np.ascontiguousarray(value[b].T).astype(BF16)
    maskT_np = None
    if not causal:
        maskT_np = np.ascontiguousarray((m2 != 0).T).astype(BF16)

    in_maps = []
    for c in range(NCORES):
        b, g = divmod(c, GROUPS)
        rows = slice(HD * g, HD * (g + 1))
        im = {
            "xqT": xT[("q", b)],
            "xkT": xT[("k", b)],
            "xvT": xT[("v", b)],
            "wqT": np.ascontiguousarray(W_q[rows, :].T).astype(BF16),
            "wkT": np.ascontiguousarray(W_k[rows, :].T).astype(BF16),
            "wvT": np.ascontiguousarray(W_v[rows, :].T).astype(BF16),
            "woT": np.ascontiguousarray(W_o[:, rows].T).astype(BF16),
        }
        if not causal:
            im["maskT"] = maskT_np
        in_maps.append(im)

    res = run_bass_kernel_spmd(nc, in_maps, core_ids=list(range(NCORES)))

    out = np.zeros((B, S, D), dtype=np.float32)
    for c in range(NCORES):
        b = c // GROUPS
        out[b] += res.results[c]["yT"].T
    return out


# revision 16
# speedup vs baseline: 1.2417x; 1.0075x over previous
"""Multi-head attention (B=2, S=2048, D=1024, H=16, causal) on 8 TRN2 cores.

Sharding: data-parallel over batch x tensor-parallel over heads (Megatron).
Core c handles batch b=c//4 and heads [4g, 4g+4) with g=c%4. Each core
computes its 4 heads' Q/K/V projections, causal attention, and its partial
output projection y_partial = attn_x @ W_o[:, cols].T; the host sums the 4
partials per batch.

Everything on-chip runs in transposed (feature x seq) layout so no
transposes are needed anywhere:
  QT/KT [256, 2048] = W @ x^T,  V [s, 4*65] with a fused ones-column,
  S^T[k, q] = KT_h.T @ QT_h,    P^T = exp(S^T/8) (ACT, scale folded),
  O^T_aug [65, q] = V_aug.T @ P^T  (row 64 = softmax denominator),
  attn^T = O^T[0:64] * bcast(1/denom),  y^T = WoT.T @ attn^T.
"""

import numpy as np
import ml_dtypes

B, S, D, H = 2, 2048, 1024, 16
DK = D // H  # 64
NCORES = 8
GROUPS = 4  # cores per batch
HPC = H // GROUPS  # heads per core = 4
HD = HPC * DK  # head dims per core = 256

BF16 = ml_dtypes.bfloat16

QCHUNK = 512  # q columns processed per softmax block
NCHUNKS = S // QCHUNK  # 4
KTILE = 128  # keys per matmul tile
NKT = S // KTILE  # 16
KSUPER = 2  # k-tiles per exp batch ([128, 1024] activations)

_prog_cache = {}


# --------------------------------------------------------------------------
# walrus workaround: this compiler build allows at most 1 semaphore wait per
# instruction; move excess waits onto NoOps inserted before the instruction.
def _split_excess_waits(nc):
    import concourse.mybir as mybir

    ctr = 0
    for f in nc.m.functions:
        for bb in f.blocks:
            out = []
            changed = False
            for inst in bb.instructions:
                si = inst.sync_info
                if si is not None and si.on_wait and len(si.on_wait) > 1:
                    waits = list(si.on_wait)
                    excess, keep = waits[:-1], waits[-1:]
                    for w in excess:
                        nop = mybir.InstNoOp(name=f"waitsplit-{ctr}", ins=[], outs=[])
                        ctr += 1
                        nop.engine = inst.engine
                        nop.sync_info = mybir.SyncInfo(on_wait=[w], on_update=[])
                        out.append(nop)
                    si.on_wait = keep
                    changed = True
                out.append(inst)
            if changed:
                bb.instructions = out
    return ctr


def _build_program(causal: bool):
    import concourse.bass as bass
    import concourse.mybir as mybir
    import concourse.tile as tile

    fp32 = mybir.dt.float32
    bf16 = mybir.dt.bfloat16

    nc = bass.Bass()

    xqT = nc.dram_tensor("xqT", [D, S], bf16, kind="ExternalInput")
    xkT = nc.dram_tensor("xkT", [D, S], bf16, kind="ExternalInput")
    xvT = nc.dram_tensor("xvT", [D, S], bf16, kind="ExternalInput")
    wqT = nc.dram_tensor("wqT", [D, HD], bf16, kind="ExternalInput")
    wkT = nc.dram_tensor("wkT", [D, HD], bf16, kind="ExternalInput")
    wvT = nc.dram_tensor("wvT", [D, HD], bf16, kind="ExternalInput")
    woT = nc.dram_tensor("woT", [HD, D], bf16, kind="ExternalInput")
    yT = nc.dram_tensor("yT", [D, S], mybir.dt.float32, kind="ExternalOutput")
    maskT = None
    if not causal:
        maskT = nc.dram_tensor("maskT", [S, S], bf16, kind="ExternalInput")

    DT = D // 128  # 8 contraction tiles for the input projections

    with tile.TileContext(nc) as tc:
        with (
            tc.tile_pool(name="wpool", bufs=1) as wpool,
            tc.tile_pool(name="res", bufs=1) as res,
            tc.tile_pool(name="xin", bufs=10) as xin,
            tc.tile_pool(name="small", bufs=1) as small,
        ):
            # ---- weights (resident); DMA issue order matters: the q-side
            # loads go first so the QT projection can start ~12us in, wo is
            # deferred to the output-projection phase.
            wq_t = [wpool.tile([128, HD], bf16, tag=f"wq{k}", name=f"wq{k}") for k in range(DT)]
            wk_t = [wpool.tile([128, HD], bf16, tag=f"wk{k}", name=f"wk{k}") for k in range(DT)]
            wv_t = [wpool.tile([128, HD], bf16, tag=f"wv{k}", name=f"wv{k}") for k in range(DT)]
            wo_t = [wpool.tile([128, D], bf16, tag=f"wo{k}", name=f"wo{k}") for k in range(HD // 128)]
            for k in range(DT):
                nc.sync.dma_start(out=wq_t[k], in_=wqT[128 * k : 128 * k + 128, :])

            # ---- resident activations ----
            # QT/KT: [128, m, S] with m in {0,1}: rows 128m..128m+127 of [256, S]
            qt = res.tile([128, 2, S], bf16, tag="qt")
            kt = res.tile([128, 2, S], bf16, tag="kt")
            # V (+ ones col): [128, ktile j, head h, 65]
            v_sb = res.tile([128, NKT, HPC, DK + 1], bf16, tag="v")
            # attention output (normalized), transposed: [128, m, S]
            at = res.tile([128, 2, S], bf16, tag="at")

            nc.vector.memset(v_sb[:, :, :, DK : DK + 1], 1.0)

            # ones row for the K=1 broadcast matmul
            ones = small.tile([128, DK], bf16, tag="ones")
            nc.vector.memset(ones, 1.0)

            # causal edge masks: m[x, y] = 1 if y >= x + off else 0
            emasks = {}
            if causal:
                for off in (0, 128, 256, 384):
                    m = small.tile([128, QCHUNK], bf16, tag=f"emask{off}")
                    nc.gpsimd.memset(m, 1.0)
                    nc.gpsimd.affine_select(
                        out=m,
                        in_=m,
                        compare_op=mybir.AluOpType.is_ge,
                        fill=0.0,
                        base=-off,
                        pattern=[[1, QCHUNK]],
                        channel_multiplier=-1,
                    )
                    emasks[off] = m

            # ---- phase 1: projections ----
            with tc.tile_pool(name="pps", bufs=3, space="PSUM") as pps:
                # QT = wqT.T @ xqT   (and KT likewise)
                for name, xdram, wdram, w_t, dst in (
                    ("q", xqT, None, wq_t, qt),
                    ("k", xkT, wkT, wk_t, kt),
                ):
                    x_t = []
                    for k in range(DT):
                        if wdram is not None:
                            nc.sync.dma_start(
                                out=w_t[k],
                                in_=wdram[128 * k : 128 * k + 128, :],
                            )
                        xt = xin.tile([128, S], bf16)
                        nc.sync.dma_start(
                            out=xt, in_=xdram[128 * k : 128 * k + 128, :]
                        )
                        x_t.append(xt)
                    for n in range(NCHUNKS):
                        for m in range(2):
                            ps = pps.tile([128, QCHUNK], fp32, tag="proj")
                            for k in range(DT):
                                nc.tensor.matmul(
                                    ps[:, :],
                                    lhsT=w_t[k][:, 128 * m : 128 * m + 128],
                                    rhs=x_t[k][:, QCHUNK * n : QCHUNK * (n + 1)],
                                    start=(k == 0),
                                    stop=(k == DT - 1),
                                )
                            nc.scalar.copy(
                                out=dst[:, m, QCHUNK * n : QCHUNK * (n + 1)],
                                in_=ps[:, :],
                            )
                # V = xvT.T @ wvT  -> [s, HD], stored per (ktile, head) with ones col
                x_t = []
                for k in range(DT):
                    nc.sync.dma_start(
                        out=wv_t[k], in_=wvT[128 * k : 128 * k + 128, :]
                    )
                    xt = xin.tile([128, S], bf16)
                    nc.sync.dma_start(out=xt, in_=xvT[128 * k : 128 * k + 128, :])
                    x_t.append(xt)
                for j in range(NKT):
                    ps = pps.tile([128, HD], fp32, tag="vproj")
                    for k in range(DT):
                        nc.tensor.matmul(
                            ps[:, :],
                            lhsT=x_t[k][:, 128 * j : 128 * j + 128],
                            rhs=wv_t[k][:, :],
                            start=(k == 0),
                            stop=(k == DT - 1),
                        )
                    nc.scalar.copy(
                        out=v_sb[:, j, :, 0:DK],
                        in_=ps.rearrange("p (h d) -> p h d", h=HPC),
                    )

            # ---- phase 2: attention ----
            with (
                tc.tile_pool(name="scps", bufs=2, space="PSUM") as scps,
                tc.tile_pool(name="pvps", bufs=4, space="PSUM") as pvps,
                tc.tile_pool(name="pt", bufs=6) as ptp,
                tc.tile_pool(name="srec", bufs=2) as srec,
                tc.tile_pool(name="mload", bufs=4) as mload,
            ):
                for c in range(NCHUNKS):
                    qs = slice(QCHUNK * c, QCHUNK * (c + 1))
                    # number of k-tiles this chunk attends to
                    nkt_c = 4 * (c + 1) if causal else NKT
                    nsup = (nkt_c + KSUPER - 1) // KSUPER
                    pvs = [
                        pvps.tile([128, QCHUNK], fp32, tag="pv", name=f"pv{c}_{h}")
                        for h in range(HPC)
                    ]
                    # interleave the 4 heads per k-super: 4 independent
                    # QK -> exp -> mask -> PV chains keep PE busy while ACT
                    # works through the exps.
                    for s_i in range(nsup):
                        jt = [
                            KSUPER * s_i + j2
                            for j2 in range(KSUPER)
                            if KSUPER * s_i + j2 < nkt_c
                        ]
                        for h in range(HPC):
                            mh = h // 2
                            ph = 64 * (h % 2)
                            pv = pvs[h]
                            sc = scps.tile(
                                [128, KSUPER * QCHUNK], fp32, tag="sc",
                                name=f"sc{c}_{s_i}_{h}",
                            )
                            for j2, j in enumerate(jt):
                                nc.tensor.matmul(
                                    sc[:, QCHUNK * j2 : QCHUNK * (j2 + 1)],
                                    lhsT=kt[ph : ph + DK, mh, 128 * j : 128 * j + 128],
                                    rhs=qt[ph : ph + DK, mh, qs],
                                    start=True,
                                    stop=True,
                                )
                            pt = ptp.tile(
                                [128, KSUPER * QCHUNK], bf16, tag="pt",
                                name=f"pt{c}_{s_i}_{h}",
                            )
                            nsc = QCHUNK * len(jt)
                            nc.scalar.activation(
                                out=pt[:, 0:nsc],
                                in_=sc[:, 0:nsc],
                                func=mybir.ActivationFunctionType.Exp,
                                scale=1.0 / np.sqrt(np.float32(DK)),
                            )
                            for j2, j in enumerate(jt):
                                pslice = pt[:, QCHUNK * j2 : QCHUNK * (j2 + 1)]
                                if causal:
                                    off = 128 * j - QCHUNK * c
                                    if off >= 0:
                                        nc.vector.tensor_mul(
                                            out=pslice, in0=pslice, in1=emasks[off]
                                        )
                                else:
                                    mt = mload.tile([128, QCHUNK], bf16, tag="mt")
                                    nc.sync.dma_start(
                                        out=mt,
                                        in_=maskT[128 * j : 128 * j + 128, qs],
                                    )
                                    nc.vector.tensor_mul(
                                        out=pslice, in0=pslice, in1=mt
                                    )
                                nc.tensor.matmul(
                                    pv[0 : DK + 1, :],
                                    lhsT=v_sb[:, j, h, :],
                                    rhs=pslice,
                                    start=(j == 0),
                                    stop=(j == nkt_c - 1),
                                )
                    # Drain each head's PV bank to SBUF immediately (frees the
                    # PSUM bank for the next chunk), then normalize entirely
                    # from SBUF off the critical path: one reciprocal per
                    # chunk, K=1 bcast matmul per head, one mul per head.
                    osbs = []
                    for h in range(HPC):
                        if not causal:
                            nc.scalar.add(
                                out=pvs[h][DK : DK + 1, :],
                                in_=pvs[h][DK : DK + 1, :],
                                add=1e-30,
                            )
                        o = srec.tile(
                            [DK + 1, QCHUNK], fp32, tag="osb", bufs=6,
                            name=f"osb{c}_{h}",
                        )
                        nc.vector.tensor_copy(o[:, :], pvs[h][0 : DK + 1, :])
                        osbs.append(o)
                    recf = srec.tile([128, QCHUNK], fp32, tag="recf")
                    recb = srec.tile([128, QCHUNK], bf16, tag="recb")
                    for h in range(HPC):
                        nc.vector.tensor_copy(
                            recf[32 * h : 32 * h + 1, :],
                            osbs[h][DK : DK + 1, :],
                        )
                    with nc.allow_low_precision(reason="softmax denom in bf16"):
                        nc.vector.reciprocal(
                            out=recb[0:97, :], in_=recf[0:97, :]
                        )
                    for h in range(HPC):
                        mh = h // 2
                        ph = 64 * (h % 2)
                        bc = pvps.tile(
                            [DK, QCHUNK], fp32, tag="pv", name=f"bc{c}_{h}"
                        )
                        nc.tensor.matmul(
                            bc[:, :],
                            lhsT=ones[32 * h : 32 * h + 1, :],
                            rhs=recb[32 * h : 32 * h + 1, :],
                            start=True,
                            stop=True,
                            tile_position=(32 * h, 0),
                        )
                        nc.vector.tensor_mul(
                            out=at[ph : ph + DK, mh, qs],
                            in0=osbs[h][0:DK, :],
                            in1=bc[:, :],
                        )

            # ---- phase 3: output projection  yT = woT.T @ attnT ----
            for k in range(HD // 128):
                nc.sync.dma_start(out=wo_t[k], in_=woT[128 * k : 128 * k + 128, :])
            with (
                tc.tile_pool(name="ops", bufs=4, space="PSUM") as ops,
                tc.tile_pool(name="ostg", bufs=4) as ostg,
            ):
                for mo in range(D // 128):
                    for n in range(NCHUNKS):
                        ps = ops.tile([128, QCHUNK], fp32, tag="out")
                        for k2 in range(HD // 128):
                            nc.tensor.matmul(
                                ps[:, :],
                                lhsT=wo_t[k2][:, 128 * mo : 128 * mo + 128],
                                rhs=at[:, k2, QCHUNK * n : QCHUNK * (n + 1)],
                                start=(k2 == 0),
                                stop=(k2 == HD // 128 - 1),
                            )
                        stg = ostg.tile([128, QCHUNK], fp32, tag="stg")
                        if (mo + n) % 2 == 0:
                            nc.scalar.copy(out=stg[:, :], in_=ps[:, :])
                        else:
                            nc.vector.tensor_copy(stg[:, :], ps[:, :])
                        nc.sync.dma_start(
                            out=yT[
                                128 * mo : 128 * mo + 128,
                                QCHUNK * n : QCHUNK * (n + 1),
                            ],
                            in_=stg[:, :],
                        )

    _split_excess_waits(nc)
    return nc


def kernel(query, key, value, mask, W_q, W_k, W_v, W_o):
    from concourse.bass_utils import run_bass_kernel_spmd

    query = np.asarray(query)
    key = np.asarray(key)
    value = np.asarray(value)
    mask = np.asarray(mask)
    W_q = np.asarray(W_q)
    W_k = np.asarray(W_k)
    W_v = np.asarray(W_v)
    W_o = np.asarray(W_o)

    m2 = mask.reshape(mask.shape[-2], mask.shape[-1])
    causal = bool(
        np.array_equal(m2 != 0, np.tril(np.ones((S, S), dtype=bool)))
    )

    if causal not in _prog_cache:
        _prog_cache[causal] = _build_program(causal)
    nc = _prog_cache[causal]

    # host-side shard prep (bf16, transposed)
    xT = {}
    for b in range(B):
        xT[("q", b)] = np.ascontiguousarray(query[b].T).astype(BF16)
        xT[("k", b)] = np.ascontiguousarray(key[b].T).astype(BF16)
        xT[("v", b)] = np.ascontiguousarray(value[b].T).astype(BF16)
    maskT_np = None
    if not causal:
        maskT_np = np.ascontiguousarray((m2 != 0).T).astype(BF16)

    in_maps = []
    for c in range(NCORES):
        b, g = divmod(c, GROUPS)
        rows = slice(HD * g, HD * (g + 1))
        im = {
            "xqT": xT[("q", b)],
            "xkT": xT[("k", b)],
            "xvT": xT[("v", b)],
            "wqT": np.ascontiguousarray(W_q[rows, :].T).astype(BF16),
            "wkT": np.ascontiguousarray(W_k[rows, :].T).astype(BF16),
            "wvT": np.ascontiguousarray(W_v[rows, :].T).astype(BF16),
            "woT": np.ascontiguousarray(W_o[:, rows].T).astype(BF16),
        }
        if not causal:
            im["maskT"] = maskT_np
        in_maps.append(im)

    res = run_bass_kernel_spmd(nc, in_maps, core_ids=list(range(NCORES)))

    out = np.zeros((B, S, D), dtype=np.float32)
    for c in range(NCORES):
        b = c // GROUPS
        out[b] += res.results[c]["yT"].T
    return out


# revision 18
# speedup vs baseline: 1.3317x; 1.0724x over previous
"""Multi-head attention (B=2, S=2048, D=1024, H=16, causal) on 8 TRN2 cores.

Sharding: data-parallel over batch x tensor-parallel over heads (Megatron).
Core c handles batch b=c//4 and heads [4g, 4g+4) with g=c%4. Each core
computes its 4 heads' Q/K/V projections, causal attention, and its partial
output projection y_partial = attn_x @ W_o[:, cols].T; the host sums the 4
partials per batch.

Everything on-chip runs in transposed (feature x seq) layout so no
transposes are needed anywhere:
  QT/KT [256, 2048] = W @ x^T,  V [s, 4*65] with a fused ones-column,
  S^T[k, q] = KT_h.T @ QT_h,    P^T = exp(S^T/8) (ACT, scale folded),
  O^T_aug [65, q] = V_aug.T @ P^T  (row 64 = softmax denominator),
  attn^T = O^T[0:64] * bcast(1/denom),  y^T = WoT.T @ attn^T.
"""

import numpy as np
import ml_dtypes

B, S, D, H = 2, 2048, 1024, 16
DK = D // H  # 64
NCORES = 8
GROUPS = 4  # cores per batch
HPC = H // GROUPS  # heads per core = 4
HD = HPC * DK  # head dims per core = 256

BF16 = ml_dtypes.bfloat16

QCHUNK = 512  # q columns processed per softmax block
NCHUNKS = S // QCHUNK  # 4
KTILE = 128  # keys per matmul tile
NKT = S // KTILE  # 16
KSUPER = 2  # k-tiles per exp batch ([128, 1024] activations)

_prog_cache = {}


# --------------------------------------------------------------------------
# walrus workaround: this compiler build allows at most 1 semaphore wait per
# instruction; move excess waits onto NoOps inserted before the instruction.
def _split_excess_waits(nc):
    import concourse.mybir as mybir

    ctr = 0
    for f in nc.m.functions:
        for bb in f.blocks:
            out = []
            changed = False
            for inst in bb.instructions:
                si = inst.sync_info
                if si is not None and si.on_wait and len(si.on_wait) > 1:
                    waits = list(si.on_wait)
                    excess, keep = waits[:-1], waits[-1:]
                    for w in excess:
                        nop = mybir.InstNoOp(name=f"waitsplit-{ctr}", ins=[], outs=[])
                        ctr += 1
                        nop.engine = inst.engine
                        nop.sync_info = mybir.SyncInfo(on_wait=[w], on_update=[])
                        out.append(nop)
                    si.on_wait = keep
                    changed = True
                out.append(inst)
            if changed:
                bb.instructions = out
    return ctr


def _build_program(causal: bool):
    import concourse.bass as bass
    import concourse.mybir as mybir
    import concourse.tile as tile

    fp32 = mybir.dt.float32
    bf16 = mybir.dt.bfloat16

    nc = bass.Bass()

    xqT = nc.dram_tensor("xqT", [D, S], bf16, kind="ExternalInput")
    xkT = nc.dram_tensor("xkT", [D, S], bf16, kind="ExternalInput")
    xvT = nc.dram_tensor("xvT", [D, S], bf16, kind="ExternalInput")
    wqT = nc.dram_tensor("wqT", [D, HD], bf16, kind="ExternalInput")
    wkT = nc.dram_tensor("wkT", [D, HD], bf16, kind="ExternalInput")
    wvT = nc.dram_tensor("wvT", [D, HD], bf16, kind="ExternalInput")
    woT = nc.dram_tensor("woT", [HD, D], bf16, kind="ExternalInput")
    yT = nc.dram_tensor("yT", [D, S], mybir.dt.float32, kind="ExternalOutput")
    maskT = None
    if not causal:
        maskT = nc.dram_tensor("maskT", [S, S], bf16, kind="ExternalInput")

    DT = D // 128  # 8 contraction tiles for the input projections

    with tile.TileContext(nc) as tc:
        with (
            tc.tile_pool(name="wpool", bufs=1) as wpool,
            tc.tile_pool(name="res", bufs=1) as res,
            tc.tile_pool(name="xin", bufs=1) as xin,
            tc.tile_pool(name="small", bufs=1) as small,
            tc.tile_pool(name="scps", bufs=2, space="PSUM") as scps,
            tc.tile_pool(name="pvps", bufs=2, space="PSUM") as pvps,
            tc.tile_pool(name="mps", bufs=2, space="PSUM") as mps,
            tc.tile_pool(name="pt", bufs=4) as ptp,
            tc.tile_pool(name="srec", bufs=2) as srec,
            tc.tile_pool(name="ostg", bufs=3) as ostg,
            tc.tile_pool(name="mload", bufs=4) as mload,
        ):
            wq_t = [wpool.tile([128, HD], bf16, tag=f"wq{k}", name=f"wq{k}") for k in range(DT)]
            wk_t = [wpool.tile([128, HD], bf16, tag=f"wk{k}", name=f"wk{k}") for k in range(DT)]
            wv_t = [wpool.tile([128, HD], bf16, tag=f"wv{k}", name=f"wv{k}") for k in range(DT)]
            wo_t = [wpool.tile([128, D], bf16, tag=f"wo{k}", name=f"wo{k}") for k in range(HD // 128)]

            # resident activations, all in (feature x seq) layout
            qt = res.tile([128, 2, S], bf16, tag="qt")
            kt = res.tile([128, 2, S], bf16, tag="kt")
            v_sb = res.tile([128, NKT, HPC, DK + 1], bf16, tag="v")
            at = res.tile([128, 2, S], bf16, tag="at")

            nc.vector.memset(v_sb[:, :, :, DK : DK + 1], 1.0)
            ones = small.tile([128, DK], bf16, tag="ones")
            nc.vector.memset(ones, 1.0)

            emasks = {}
            if causal:
                for off in (0, 128, 256, 384):
                    m = small.tile([128, QCHUNK], bf16, tag=f"emask{off}", name=f"emask{off}")
                    nc.gpsimd.memset(m, 1.0)
                    nc.gpsimd.affine_select(
                        out=m,
                        in_=m,
                        compare_op=mybir.AluOpType.is_ge,
                        fill=0.0,
                        base=-off,
                        pattern=[[1, QCHUNK]],
                        channel_multiplier=-1,
                    )
                    emasks[off] = m

            # ---- input DMAs (issue order = arrival order) ----
            xq_t, xk_t, xv_t = [], [], []
            for w_t, wdram, x_t, xdram in (
                (wq_t, wqT, xq_t, xqT),
                (wk_t, wkT, xk_t, xkT),
                (wv_t, wvT, xv_t, xvT),
            ):
                for k in range(DT):
                    nc.sync.dma_start(
                        out=w_t[k], in_=wdram[128 * k : 128 * k + 128, :]
                    )
                    xt = xin.tile([128, S], bf16, tag=f"x{id(x_t)}_{k}", name=f"x{k}")
                    nc.sync.dma_start(
                        out=xt, in_=xdram[128 * k : 128 * k + 128, :]
                    )
                    x_t.append(xt)
            for k in range(HD // 128):
                nc.sync.dma_start(out=wo_t[k], in_=woT[128 * k : 128 * k + 128, :])

            # ---- projection-group emitters ----
            def proj_group(w_t, x_t, dst, n, m):
                ps = mps.tile([128, QCHUNK], fp32, tag="misc", name=f"pj{n}_{m}")
                for k in range(DT):
                    nc.tensor.matmul(
                        ps[:, :],
                        lhsT=w_t[k][:, 128 * m : 128 * m + 128],
                        rhs=x_t[k][:, QCHUNK * n : QCHUNK * (n + 1)],
                        start=(k == 0),
                        stop=(k == DT - 1),
                    )
                nc.scalar.copy(
                    out=dst[:, m, QCHUNK * n : QCHUNK * (n + 1)], in_=ps[:, :]
                )

            def v_group(j):
                ps = mps.tile([128, HD], fp32, tag="misc", name=f"vj{j}")
                for k in range(DT):
                    nc.tensor.matmul(
                        ps[:, :],
                        lhsT=xv_t[k][:, 128 * j : 128 * j + 128],
                        rhs=wv_t[k][:, :],
                        start=(k == 0),
                        stop=(k == DT - 1),
                    )
                nc.scalar.copy(
                    out=v_sb[:, j, :, 0:DK],
                    in_=ps.rearrange("p (h d) -> p h d", h=HPC),
                )

            # chunk 0 prerequisites up front
            for m in range(2):
                proj_group(wq_t, xq_t, qt, 0, m)
            for m in range(2):
                proj_group(wk_t, xk_t, kt, 0, m)
            for j in range(NKT):
                v_group(j)

            # remaining QT/KT chunks stream into the attention loop slots
            from collections import deque

            pending = deque()
            for n in range(1, NCHUNKS):
                for m in range(2):
                    pending.append(lambda n=n, m=m: proj_group(wq_t, xq_t, qt, n, m))
                for m in range(2):
                    pending.append(lambda n=n, m=m: proj_group(wk_t, xk_t, kt, n, m))

            # ---- fused attention + streamed projections + output proj ----
            for c in range(NCHUNKS):
                qs = slice(QCHUNK * c, QCHUNK * (c + 1))
                nkt_c = 4 * (c + 1) if causal else NKT
                nsup = (nkt_c + KSUPER - 1) // KSUPER
                nslots = 2 * nsup
                # how many pending proj groups to emit per slot this chunk:
                # chunk c may only emit projections for chunks > c that are
                # already loadable; chunk0 must finish n=1 before chunk 1.
                want = 0
                if c == 0:
                    want = 8  # n=1 and n=2 q/k groups during chunk 0
                elif c == 1:
                    want = len(pending)  # the rest (n=3)
                per_slot = (want + nslots - 1) // nslots if want else 0

                osbs = {}
                for pair in range(2):
                    hs = (2 * pair, 2 * pair + 1)
                    pvs = {
                        h: pvps.tile(
                            [DK + 1, QCHUNK], fp32, tag="pv", name=f"pv{c}_{h}"
                        )
                        for h in hs
                    }
                    for s_i in range(nsup):
                        emitted = 0
                        while pending and emitted < per_slot:
                            pending.popleft()()
                            emitted += 1
                        jt = [
                            KSUPER * s_i + j2
                            for j2 in range(KSUPER)
                            if KSUPER * s_i + j2 < nkt_c
                        ]
                        scs = {
                            h: scps.tile(
                                [128, KSUPER * QCHUNK], fp32, tag="sc",
                                name=f"sc{c}_{s_i}_{h}",
                            )
                            for h in hs
                        }
                        # adjacent QKs alternate PE row groups (rows 0-63 for
                        # even heads, 64-127 for odd) -> run concurrently
                        for j2, j in enumerate(jt):
                            for h in hs:
                                mh = h // 2
                                ph = 64 * (h % 2)
                                nc.tensor.matmul(
                                    scs[h][:, QCHUNK * j2 : QCHUNK * (j2 + 1)],
                                    lhsT=kt[ph : ph + DK, mh, 128 * j : 128 * j + 128],
                                    rhs=qt[ph : ph + DK, mh, qs],
                                    start=True,
                                    stop=True,
                                )
                        for h in hs:
                            pt = ptp.tile(
                                [128, KSUPER * QCHUNK], bf16, tag="pt",
                                name=f"pt{c}_{s_i}_{h}",
                            )
                            nsc = QCHUNK * len(jt)
                            nc.scalar.activation(
                                out=pt[:, 0:nsc],
                                in_=scs[h][:, 0:nsc],
                                func=mybir.ActivationFunctionType.Exp,
                                scale=1.0 / np.sqrt(np.float32(DK)),
                            )
                            for j2, j in enumerate(jt):
                                pslice = pt[:, QCHUNK * j2 : QCHUNK * (j2 + 1)]
                                if causal:
                                    off = 128 * j - QCHUNK * c
                                    if off >= 0:
                                        nc.vector.tensor_mul(
                                            out=pslice, in0=pslice, in1=emasks[off]
                                        )
                                else:
                                    mt = mload.tile(
                                        [128, QCHUNK], bf16, tag="mt",
                                        name=f"mt{c}_{s_i}_{h}_{j2}",
                                    )
                                    nc.sync.dma_start(
                                        out=mt,
                                        in_=maskT[128 * j : 128 * j + 128, qs],
                                    )
                                    nc.vector.tensor_mul(
                                        out=pslice, in0=pslice, in1=mt
                                    )
                                nc.tensor.matmul(
                                    pvs[h][0 : DK + 1, :],
                                    lhsT=v_sb[:, j, h, :],
                                    rhs=pslice,
                                    start=(j == 0),
                                    stop=(j == nkt_c - 1),
                                )
                    # drain this pair's PV banks to SBUF
                    for h in hs:
                        if not causal:
                            nc.scalar.add(
                                out=pvs[h][DK : DK + 1, :],
                                in_=pvs[h][DK : DK + 1, :],
                                add=1e-30,
                            )
                        o = srec.tile(
                            [DK + 1, QCHUNK], fp32, tag="osb", bufs=6,
                            name=f"osb{c}_{h}",
                        )
                        nc.vector.tensor_copy(o[:, :], pvs[h][0 : DK + 1, :])
                        osbs[h] = o

                # normalize all 4 heads of this chunk (off critical path)
                recf = srec.tile([128, QCHUNK], fp32, tag="recf", name=f"recf{c}")
                recb = srec.tile([128, QCHUNK], bf16, tag="recb", name=f"recb{c}")
                for h in range(HPC):
                    nc.vector.tensor_copy(
                        recf[32 * h : 32 * h + 1, :],
                        osbs[h][DK : DK + 1, :],
                    )
                with nc.allow_low_precision(reason="softmax denom in bf16"):
                    nc.vector.reciprocal(out=recb[0:97, :], in_=recf[0:97, :])
                for h in range(HPC):
                    mh = h // 2
                    ph = 64 * (h % 2)
                    bc = mps.tile([DK, QCHUNK], fp32, tag="misc", name=f"bc{c}_{h}")
                    nc.tensor.matmul(
                        bc[:, :],
                        lhsT=ones[32 * h : 32 * h + 1, :],
                        rhs=recb[32 * h : 32 * h + 1, :],
                        start=True,
                        stop=True,
                        tile_position=(32 * h, 0),
                    )
                    nc.vector.tensor_mul(
                        out=at[ph : ph + DK, mh, qs],
                        in0=osbs[h][0:DK, :],
                        in1=bc[:, :],
                    )

                # output projection for this chunk: yT[:, qs]
                for mo in range(D // 128):
                    ps = mps.tile(
                        [128, QCHUNK], fp32, tag="misc", name=f"op{c}_{mo}"
                    )
                    for k2 in range(HD // 128):
                        nc.tensor.matmul(
                            ps[:, :],
                            lhsT=wo_t[k2][:, 128 * mo : 128 * mo + 128],
                            rhs=at[:, k2, qs],
                            start=(k2 == 0),
                            stop=(k2 == HD // 128 - 1),
                        )
                    stg = ostg.tile(
                        [128, QCHUNK], fp32, tag="stg", name=f"stg{c}_{mo}"
                    )
                    if mo % 2 == 0:
                        nc.scalar.copy(out=stg[:, :], in_=ps[:, :])
                    else:
                        nc.vector.tensor_copy(stg[:, :], ps[:, :])
                    nc.sync.dma_start(
                        out=yT[128 * mo : 128 * mo + 128, qs],
                        in_=stg[:, :],
                    )

    _split_excess_waits(nc)
    return nc


def kernel(query, key, value, mask, W_q, W_k, W_v, W_o):
    from concourse.bass_utils import run_bass_kernel_spmd

    query = np.asarray(query)
    key = np.asarray(key)
    value = np.asarray(value)
    mask = np.asarray(mask)
    W_q = np.asarray(W_q)
    W_k = np.asarray(W_k)
    W_v = np.asarray(W_v)
    W_o = np.asarray(W_o)

    m2 = mask.reshape(mask.shape[-2], mask.shape[-1])
    causal = bool(
        np.array_equal(m2 != 0, np.tril(np.ones((S, S), dtype=bool)))
    )

    if causal not in _prog_cache:
        _prog_cache[causal] = _build_program(causal)
    nc = _prog_cache[causal]

    # host-side shard prep (bf16, transposed)
    xT = {}
    for b in range(B):
        xT[("q", b)] = np.ascontiguousarray(query[b].T).astype(BF16)
        xT[("k", b)] = np.ascontiguousarray(key[b].T).astype(BF16)
        xT[("v", b)] = np.ascontiguousarray(value[b].T).astype(BF16)
    maskT_np = None
    if not causal:
        maskT_np = np.ascontiguousarray((m2 != 0).T).astype(BF16)

    in_maps = []
    for c in range(NCORES):
        b, g = divmod(c, GROUPS)
        rows = slice(HD * g, HD * (g + 1))
        im = {
            "xqT": xT[("q", b)],
            "xkT": xT[("k", b)],
            "xvT": xT[("v", b)],
            "wqT": np.ascontiguousarray(W_q[rows, :].T).astype(BF16),
            "wkT": np.ascontiguousarray(W_k[rows, :].T).astype(BF16),
            "wvT": np.ascontiguousarray(W_v[rows, :].T).astype(BF16),
            "woT": np.ascontiguousarray(W_o[:, rows].T).astype(BF16),
        }
        if not causal:
            im["maskT"] = maskT_np
        in_maps.append(im)

    res = run_bass_kernel_spmd(nc, in_maps, core_ids=list(range(NCORES)))

    out = np.zeros((B, S, D), dtype=np.float32)
    for c in range(NCORES):
        b = c // GROUPS
        out[b] += res.results[c]["yT"].T
    return out


# revision 19
# speedup vs baseline: 1.3685x; 1.0277x over previous
"""Multi-head attention (B=2, S=2048, D=1024, H=16, causal) on 8 TRN2 cores.

Sharding: data-parallel over batch x tensor-parallel over heads (Megatron).
Core c handles batch b=c//4 and heads [4g, 4g+4) with g=c%4. Each core
computes its 4 heads' Q/K/V projections, causal attention, and its partial
output projection y_partial = attn_x @ W_o[:, cols].T; the host sums the 4
partials per batch.

Everything on-chip runs in transposed (feature x seq) layout so no
transposes are needed anywhere:
  QT/KT [256, 2048] = W @ x^T,  V [s, 4*65] with a fused ones-column,
  S^T[k, q] = KT_h.T @ QT_h,    P^T = exp(S^T/8) (ACT, scale folded),
  O^T_aug [65, q] = V_aug.T @ P^T  (row 64 = softmax denominator),
  attn^T = O^T[0:64] * bcast(1/denom),  y^T = WoT.T @ attn^T.
"""

import numpy as np
import ml_dtypes

B, S, D, H = 2, 2048, 1024, 16
DK = D // H  # 64
NCORES = 8
GROUPS = 4  # cores per batch
HPC = H // GROUPS  # heads per core = 4
HD = HPC * DK  # head dims per core = 256

BF16 = ml_dtypes.bfloat16

QCHUNK = 512  # q columns processed per softmax block
NCHUNKS = S // QCHUNK  # 4
KTILE = 128  # keys per matmul tile
NKT = S // KTILE  # 16
KSUPER = 2  # k-tiles per exp batch ([128, 1024] activations)

_prog_cache = {}


# --------------------------------------------------------------------------
# walrus workaround: this compiler build allows at most 1 semaphore wait per
# instruction; move excess waits onto NoOps inserted before the instruction.
def _split_excess_waits(nc):
    import concourse.mybir as mybir

    ctr = 0
    for f in nc.m.functions:
        for bb in f.blocks:
            out = []
            changed = False
            for inst in bb.instructions:
                si = inst.sync_info
                if si is not None and si.on_wait and len(si.on_wait) > 1:
                    waits = list(si.on_wait)
                    excess, keep = waits[:-1], waits[-1:]
                    for w in excess:
                        nop = mybir.InstNoOp(name=f"waitsplit-{ctr}", ins=[], outs=[])
                        ctr += 1
                        nop.engine = inst.engine
                        nop.sync_info = mybir.SyncInfo(on_wait=[w], on_update=[])
                        out.append(nop)
                    si.on_wait = keep
                    changed = True
                out.append(inst)
            if changed:
                bb.instructions = out
    return ctr


def _build_program(causal: bool):
    import concourse.bass as bass
    import concourse.mybir as mybir
    import concourse.tile as tile

    fp32 = mybir.dt.float32
    bf16 = mybir.dt.bfloat16

    nc = bass.Bass()

    xqT = nc.dram_tensor("xqT", [D, S], bf16, kind="ExternalInput")
    xkT = nc.dram_tensor("xkT", [D, S], bf16, kind="ExternalInput")
    xvT = nc.dram_tensor("xvT", [D, S], bf16, kind="ExternalInput")
    wqT = nc.dram_tensor("wqT", [D, HD], bf16, kind="ExternalInput")
    wkT = nc.dram_tensor("wkT", [D, HD], bf16, kind="ExternalInput")
    wvT = nc.dram_tensor("wvT", [D, HD], bf16, kind="ExternalInput")
    woT = nc.dram_tensor("woT", [HD, D], bf16, kind="ExternalInput")
    yT = nc.dram_tensor("yT", [D, S], mybir.dt.float32, kind="ExternalOutput")
    maskT = None
    if not causal:
        maskT = nc.dram_tensor("maskT", [S, S], bf16, kind="ExternalInput")

    DT = D // 128  # 8 contraction tiles for the input projections

    with tile.TileContext(nc) as tc:
        with (
            tc.tile_pool(name="wpool", bufs=1) as wpool,
            tc.tile_pool(name="res", bufs=1) as res,
            tc.tile_pool(name="xin", bufs=1) as xin,
            tc.tile_pool(name="small", bufs=1) as small,
            tc.tile_pool(name="scps", bufs=2, space="PSUM") as scps,
            tc.tile_pool(name="pvps", bufs=2, space="PSUM") as pvps,
            tc.tile_pool(name="mps", bufs=2, space="PSUM") as mps,
            tc.tile_pool(name="pt", bufs=4) as ptp,
            tc.tile_pool(name="srec", bufs=2) as srec,
            tc.tile_pool(name="ostg", bufs=3) as ostg,
            tc.tile_pool(name="mload", bufs=4) as mload,
        ):
            wq_t = [wpool.tile([128, HD], bf16, tag=f"wq{k}", name=f"wq{k}") for k in range(DT)]
            wk_t = [wpool.tile([128, HD], bf16, tag=f"wk{k}", name=f"wk{k}") for k in range(DT)]
            wv_t = [wpool.tile([128, HD], bf16, tag=f"wv{k}", name=f"wv{k}") for k in range(DT)]
            wo_t = [wpool.tile([128, D], bf16, tag=f"wo{k}", name=f"wo{k}") for k in range(HD // 128)]

            # resident activations, all in (feature x seq) layout
            qt = res.tile([128, 2, S], bf16, tag="qt")
            kt = res.tile([128, 2, S], bf16, tag="kt")
            v_sb = res.tile([128, NKT, HPC, DK + 1], bf16, tag="v")
            at = res.tile([128, 2, S], bf16, tag="at")

            nc.vector.memset(v_sb[:, :, :, DK : DK + 1], 1.0)
            ones = small.tile([128, DK], bf16, tag="ones")
            nc.vector.memset(ones, 1.0)

            emasks = {}
            if causal:
                for off in (0, 128, 256, 384):
                    m = small.tile([128, QCHUNK], bf16, tag=f"emask{off}", name=f"emask{off}")
                    nc.gpsimd.memset(m, 1.0)
                    nc.gpsimd.affine_select(
                        out=m,
                        in_=m,
                        compare_op=mybir.AluOpType.is_ge,
                        fill=0.0,
                        base=-off,
                        pattern=[[1, QCHUNK]],
                        channel_multiplier=-1,
                    )
                    emasks[off] = m

            # ---- input DMAs (issue order = arrival order) ----
            xq_t, xk_t, xv_t = [], [], []
            for w_t, wdram, x_t, xdram in (
                (wq_t, wqT, xq_t, xqT),
                (wk_t, wkT, xk_t, xkT),
                (wv_t, wvT, xv_t, xvT),
            ):
                for k in range(DT):
                    nc.sync.dma_start(
                        out=w_t[k], in_=wdram[128 * k : 128 * k + 128, :]
                    )
                    xt = xin.tile([128, S], bf16, tag=f"x{id(x_t)}_{k}", name=f"x{k}")
                    nc.sync.dma_start(
                        out=xt, in_=xdram[128 * k : 128 * k + 128, :]
                    )
                    x_t.append(xt)
            for k in range(HD // 128):
                nc.sync.dma_start(out=wo_t[k], in_=woT[128 * k : 128 * k + 128, :])

            # ---- projection-group emitters ----
            def proj_group(w_t, x_t, dst, n, m):
                ps = mps.tile([128, QCHUNK], fp32, tag="misc", name=f"pj{n}_{m}")
                for k in range(DT):
                    nc.tensor.matmul(
                        ps[:, :],
                        lhsT=w_t[k][:, 128 * m : 128 * m + 128],
                        rhs=x_t[k][:, QCHUNK * n : QCHUNK * (n + 1)],
                        start=(k == 0),
                        stop=(k == DT - 1),
                    )
                if (n + m) % 2 == 0:
                    nc.scalar.copy(
                        out=dst[:, m, QCHUNK * n : QCHUNK * (n + 1)],
                        in_=ps[:, :],
                    )
                else:
                    nc.vector.tensor_copy(
                        dst[:, m, QCHUNK * n : QCHUNK * (n + 1)], ps[:, :]
                    )

            def v_group(j):
                ps = mps.tile([128, HD], fp32, tag="misc", name=f"vj{j}")
                for k in range(DT):
                    nc.tensor.matmul(
                        ps[:, :],
                        lhsT=xv_t[k][:, 128 * j : 128 * j + 128],
                        rhs=wv_t[k][:, :],
                        start=(k == 0),
                        stop=(k == DT - 1),
                    )
                if j % 2 == 0:
                    nc.scalar.copy(
                        out=v_sb[:, j, :, 0:DK],
                        in_=ps.rearrange("p (h d) -> p h d", h=HPC),
                    )
                else:
                    nc.vector.tensor_copy(
                        v_sb[:, j, :, 0:DK],
                        ps.rearrange("p (h d) -> p h d", h=HPC),
                    )

            # chunk 0 prerequisites up front
            for m in range(2):
                proj_group(wq_t, xq_t, qt, 0, m)
            for m in range(2):
                proj_group(wk_t, xk_t, kt, 0, m)
            for j in range(4):
                v_group(j)

            # later QT/KT chunks, V tiles, and output projections stream into
            # the attention loop slots: queues[c] pops during chunk c.
            from collections import deque

            queues = [deque() for _ in range(NCHUNKS)]
            for n in range(1, NCHUNKS):
                for m in range(2):
                    queues[n - 1].append(
                        lambda n=n, m=m: proj_group(wq_t, xq_t, qt, n, m)
                    )
                for m in range(2):
                    queues[n - 1].append(
                        lambda n=n, m=m: proj_group(wk_t, xk_t, kt, n, m)
                    )
                for j in range(4 * n, 4 * n + 4):
                    queues[n - 1].append(lambda j=j: v_group(j))

            # ---- fused attention + streamed projections + output proj ----
            for c in range(NCHUNKS):
                qs = slice(QCHUNK * c, QCHUNK * (c + 1))
                nkt_c = 4 * (c + 1) if causal else NKT
                nsup = (nkt_c + KSUPER - 1) // KSUPER
                nslots = 2 * nsup
                pending = queues[c]
                per_slot = (len(pending) + nslots - 1) // max(1, nslots)

                osbs = {}
                for pair in range(2):
                    hs = (2 * pair, 2 * pair + 1)
                    pvs = {
                        h: pvps.tile(
                            [DK + 1, QCHUNK], fp32, tag="pv", name=f"pv{c}_{h}"
                        )
                        for h in hs
                    }
                    for s_i in range(nsup):
                        emitted = 0
                        while pending and emitted < per_slot:
                            pending.popleft()()
                            emitted += 1
                        jt = [
                            KSUPER * s_i + j2
                            for j2 in range(KSUPER)
                            if KSUPER * s_i + j2 < nkt_c
                        ]
                        scs = {
                            h: scps.tile(
                                [128, KSUPER * QCHUNK], fp32, tag="sc",
                                name=f"sc{c}_{s_i}_{h}",
                            )
                            for h in hs
                        }
                        # adjacent QKs alternate PE row groups (rows 0-63 for
                        # even heads, 64-127 for odd) -> run concurrently
                        for j2, j in enumerate(jt):
                            for h in hs:
                                mh = h // 2
                                ph = 64 * (h % 2)
                                nc.tensor.matmul(
                                    scs[h][:, QCHUNK * j2 : QCHUNK * (j2 + 1)],
                                    lhsT=kt[ph : ph + DK, mh, 128 * j : 128 * j + 128],
                                    rhs=qt[ph : ph + DK, mh, qs],
                                    start=True,
                                    stop=True,
                                )
                        for h in hs:
                            pt = ptp.tile(
                                [128, KSUPER * QCHUNK], bf16, tag="pt",
                                name=f"pt{c}_{s_i}_{h}",
                            )
                            nsc = QCHUNK * len(jt)
                            nc.scalar.activation(
                                out=pt[:, 0:nsc],
                                in_=scs[h][:, 0:nsc],
                                func=mybir.ActivationFunctionType.Exp,
                                scale=1.0 / np.sqrt(np.float32(DK)),
                            )
                            for j2, j in enumerate(jt):
                                pslice = pt[:, QCHUNK * j2 : QCHUNK * (j2 + 1)]
                                if causal:
                                    off = 128 * j - QCHUNK * c
                                    if off >= 0:
                                        nc.vector.tensor_mul(
                                            out=pslice, in0=pslice, in1=emasks[off]
                                        )
                                else:
                                    mt = mload.tile(
                                        [128, QCHUNK], bf16, tag="mt",
                                        name=f"mt{c}_{s_i}_{h}_{j2}",
                                    )
                                    nc.sync.dma_start(
                                        out=mt,
                                        in_=maskT[128 * j : 128 * j + 128, qs],
                                    )
                                    nc.vector.tensor_mul(
                                        out=pslice, in0=pslice, in1=mt
                                    )
                                nc.tensor.matmul(
                                    pvs[h][0 : DK + 1, :],
                                    lhsT=v_sb[:, j, h, :],
                                    rhs=pslice,
                                    start=(j == 0),
                                    stop=(j == nkt_c - 1),
                                )
                    # drain this pair's PV banks to SBUF
                    for h in hs:
                        if not causal:
                            nc.scalar.add(
                                out=pvs[h][DK : DK + 1, :],
                                in_=pvs[h][DK : DK + 1, :],
                                add=1e-30,
                            )
                        o = srec.tile(
                            [DK + 1, QCHUNK], fp32, tag="osb", bufs=6,
                            name=f"osb{c}_{h}",
                        )
                        nc.vector.tensor_copy(o[:, :], pvs[h][0 : DK + 1, :])
                        osbs[h] = o

                # normalize all 4 heads of this chunk (off critical path)
                recf = srec.tile([128, QCHUNK], fp32, tag="recf", name=f"recf{c}")
                recb = srec.tile([128, QCHUNK], bf16, tag="recb", name=f"recb{c}")
                for h in range(HPC):
                    nc.vector.tensor_copy(
                        recf[32 * h : 32 * h + 1, :],
                        osbs[h][DK : DK + 1, :],
                    )
                with nc.allow_low_precision(reason="softmax denom in bf16"):
                    nc.vector.reciprocal(out=recb[0:97, :], in_=recf[0:97, :])
                for h in range(HPC):
                    mh = h // 2
                    ph = 64 * (h % 2)
                    bc = mps.tile([DK, QCHUNK], fp32, tag="misc", name=f"bc{c}_{h}")
                    nc.tensor.matmul(
                        bc[:, :],
                        lhsT=ones[32 * h : 32 * h + 1, :],
                        rhs=recb[32 * h : 32 * h + 1, :],
                        start=True,
                        stop=True,
                        tile_position=(32 * h, 0),
                    )
                    nc.vector.tensor_mul(
                        out=at[ph : ph + DK, mh, qs],
                        in0=osbs[h][0:DK, :],
                        in1=bc[:, :],
                    )

                # output projection for this chunk streams into the next
                # chunk's slots (chunk 3's runs right here).
                def op_group(c, mo, qs=qs):
                    ps = mps.tile(
                        [128, QCHUNK], fp32, tag="misc", name=f"op{c}_{mo}"
                    )
                    for k2 in range(HD // 128):
                        nc.tensor.matmul(
                            ps[:, :],
                            lhsT=wo_t[k2][:, 128 * mo : 128 * mo + 128],
                            rhs=at[:, k2, qs],
                            start=(k2 == 0),
                            stop=(k2 == HD // 128 - 1),
                        )
                    stg = ostg.tile(
                        [128, QCHUNK], fp32, tag="stg", name=f"stg{c}_{mo}"
                    )
                    if mo % 2 == 0:
                        nc.scalar.copy(out=stg[:, :], in_=ps[:, :])
                    else:
                        nc.vector.tensor_copy(stg[:, :], ps[:, :])
                    nc.sync.dma_start(
                        out=yT[128 * mo : 128 * mo + 128, qs],
                        in_=stg[:, :],
                    )

                for mo in range(D // 128):
                    if c + 1 < NCHUNKS:
                        queues[c + 1].append(lambda c=c, mo=mo: op_group(c, mo))
                    else:
                        op_group(c, mo)

    _split_excess_waits(nc)
    return nc


def kernel(query, key, value, mask, W_q, W_k, W_v, W_o):
    from concourse.bass_utils import run_bass_kernel_spmd

    query = np.asarray(query)
    key = np.asarray(key)
    value = np.asarray(value)
    mask = np.asarray(mask)
    W_q = np.asarray(W_q)
    W_k = np.asarray(W_k)
    W_v = np.asarray(W_v)
    W_o = np.asarray(W_o)

    m2 = mask.reshape(mask.shape[-2], mask.shape[-1])
    causal = bool(
        np.array_equal(m2 != 0, np.tril(np.ones((S, S), dtype=bool)))
    )

    if causal not in _prog_cache:
        _prog_cache[causal] = _build_program(causal)
    nc = _prog_cache[causal]

    # host-side shard prep (bf16, transposed)
    xT = {}
    for b in range(B):
        xT[("q", b)] = np.ascontiguousarray(query[b].T).astype(BF16)
        xT[("k", b)] = np.ascontiguousarray(key[b].T).astype(BF16)
        xT[("v", b)] = np.ascontiguousarray(value[b].T).astype(BF16)
    maskT_np = None
    if not causal:
        maskT_np = np.ascontiguousarray((m2 != 0).T).astype(BF16)

    in_maps = []
    for c in range(NCORES):
        b, g = divmod(c, GROUPS)
        rows = slice(HD * g, HD * (g + 1))
        im = {
            "xqT": xT[("q", b)],
            "xkT": xT[("k", b)],
            "xvT": xT[("v", b)],
            "wqT": np.ascontiguousarray(W_q[rows, :].T).astype(BF16),
            "wkT": np.ascontiguousarray(W_k[rows, :].T).astype(BF16),
            "wvT": np.ascontiguousarray(W_v[rows, :].T).astype(BF16),
            "woT": np.ascontiguousarray(W_o[:, rows].T).astype(BF16),
        }
        if not causal:
            im["maskT"] = maskT_np
        in_maps.append(im)

    res = run_bass_kernel_spmd(nc, in_maps, core_ids=list(range(NCORES)))

    out = np.zeros((B, S, D), dtype=np.float32)
    for c in range(NCORES):
        b = c // GROUPS
        out[b] += res.results[c]["yT"].T
    return out


# revision 22
# speedup vs baseline: 1.4156x; 1.0344x over previous
"""Multi-head attention (B=2, S=2048, D=1024, H=16, causal) on 8 TRN2 cores.

Sharding: data-parallel over batch x tensor-parallel over heads (Megatron).
Core c handles batch b=c//4 and heads [4g, 4g+4) with g=c%4. Each core
computes its 4 heads' Q/K/V projections, causal attention, and its partial
output projection y_partial = attn_x @ W_o[:, cols].T; the host sums the 4
partials per batch.

Everything on-chip runs in transposed (feature x seq) layout so no
transposes are needed anywhere:
  QT/KT [256, 2048] = W @ x^T,  V [s, 4*65] with a fused ones-column,
  S^T[k, q] = KT_h.T @ QT_h,    P^T = exp(S^T/8) (ACT, scale folded),
  O^T_aug [65, q] = V_aug.T @ P^T  (row 64 = softmax denominator),
  attn^T = O^T[0:64] * bcast(1/denom),  y^T = WoT.T @ attn^T.
"""

import numpy as np
import ml_dtypes

B, S, D, H = 2, 2048, 1024, 16
DK = D // H  # 64
NCORES = 8
GROUPS = 4  # cores per batch
HPC = H // GROUPS  # heads per core = 4
HD = HPC * DK  # head dims per core = 256

BF16 = ml_dtypes.bfloat16

QCHUNK = 512  # q columns processed per softmax block
NCHUNKS = S // QCHUNK  # 4
KTILE = 128  # keys per matmul tile
NKT = S // KTILE  # 16
KSUPER = 2  # k-tiles per exp batch ([128, 1024] activations)

_prog_cache = {}


# --------------------------------------------------------------------------
# walrus workaround: this compiler build allows at most 1 semaphore wait per
# instruction; move excess waits onto NoOps inserted before the instruction.
def _split_excess_waits(nc):
    import concourse.mybir as mybir

    ctr = 0
    for f in nc.m.functions:
        for bb in f.blocks:
            out = []
            changed = False
            for inst in bb.instructions:
                si = inst.sync_info
                if si is not None and si.on_wait and len(si.on_wait) > 1:
                    waits = list(si.on_wait)
                    excess, keep = waits[:-1], waits[-1:]
                    for w in excess:
                        nop = mybir.InstNoOp(name=f"waitsplit-{ctr}", ins=[], outs=[])
                        ctr += 1
                        nop.engine = inst.engine
                        nop.sync_info = mybir.SyncInfo(on_wait=[w], on_update=[])
                        out.append(nop)
                    si.on_wait = keep
                    changed = True
                out.append(inst)
            if changed:
                bb.instructions = out
    return ctr


def _build_program(causal: bool):
    import concourse.bass as bass
    import concourse.mybir as mybir
    import concourse.tile as tile

    fp32 = mybir.dt.float32
    bf16 = mybir.dt.bfloat16

    nc = bass.Bass()

    xqT = nc.dram_tensor("xqT", [D, S], bf16, kind="ExternalInput")
    xkT = nc.dram_tensor("xkT", [D, S], bf16, kind="ExternalInput")
    xvT = nc.dram_tensor("xvT", [D, S], bf16, kind="ExternalInput")
    wqT = nc.dram_tensor("wqT", [D, HD], bf16, kind="ExternalInput")
    wkT = nc.dram_tensor("wkT", [D, HD], bf16, kind="ExternalInput")
    wvT = nc.dram_tensor("wvT", [D, HD], bf16, kind="ExternalInput")
    woT = nc.dram_tensor("woT", [HD, D], bf16, kind="ExternalInput")
    yT = nc.dram_tensor("yT", [D, S], mybir.dt.float32, kind="ExternalOutput")
    maskT = None
    if not causal:
        maskT = nc.dram_tensor("maskT", [S, S], bf16, kind="ExternalInput")

    DT = D // 128  # 8 contraction tiles for the input projections

    with tile.TileContext(nc) as tc:
        with (
            tc.tile_pool(name="wpool", bufs=1) as wpool,
            tc.tile_pool(name="res", bufs=1) as res,
            tc.tile_pool(name="xin", bufs=1) as xin,
            tc.tile_pool(name="small", bufs=1) as small,
            tc.tile_pool(name="scps", bufs=2, space="PSUM") as scps,
            tc.tile_pool(name="pvps", bufs=2, space="PSUM") as pvps,
            tc.tile_pool(name="mps", bufs=2, space="PSUM") as mps,
            tc.tile_pool(name="pt", bufs=4) as ptp,
            tc.tile_pool(name="srec", bufs=2) as srec,
            tc.tile_pool(name="ostg", bufs=3) as ostg,
            tc.tile_pool(name="mload", bufs=4) as mload,
        ):
            wq_t = [wpool.tile([128, HD], bf16, tag=f"wq{k}", name=f"wq{k}") for k in range(DT)]
            wk_t = [wpool.tile([128, HD], bf16, tag=f"wk{k}", name=f"wk{k}") for k in range(DT)]
            wv_t = [wpool.tile([128, HD], bf16, tag=f"wv{k}", name=f"wv{k}") for k in range(DT)]
            wo_t = [wpool.tile([128, D], bf16, tag=f"wo{k}", name=f"wo{k}") for k in range(HD // 128)]

            # resident activations, all in (feature x seq) layout
            qt = res.tile([128, 2, S], bf16, tag="qt")
            kt = res.tile([128, 2, S], bf16, tag="kt")
            v_sb = res.tile([128, NKT, HPC, DK + 1], bf16, tag="v")
            at = res.tile([128, 2, S], bf16, tag="at")

            nc.vector.memset(v_sb[:, :, :, DK : DK + 1], 1.0)
            ones = small.tile([128, DK], bf16, tag="ones")
            nc.vector.memset(ones, 1.0)

            emasks = {}
            if causal:
                for off in (0, 128, 256, 384):
                    m = small.tile([128, QCHUNK], bf16, tag=f"emask{off}", name=f"emask{off}")
                    nc.gpsimd.memset(m, 1.0)
                    nc.gpsimd.affine_select(
                        out=m,
                        in_=m,
                        compare_op=mybir.AluOpType.is_ge,
                        fill=0.0,
                        base=-off,
                        pattern=[[1, QCHUNK]],
                        channel_multiplier=-1,
                    )
                    emasks[off] = m

            # ---- input DMAs (issue order = arrival order) ----
            xq_t, xk_t, xv_t = [], [], []
            for w_t, wdram, x_t, xdram in (
                (wq_t, wqT, xq_t, xqT),
                (wk_t, wkT, xk_t, xkT),
                (wv_t, wvT, xv_t, xvT),
            ):
                for k in range(DT):
                    nc.sync.dma_start(
                        out=w_t[k], in_=wdram[128 * k : 128 * k + 128, :]
                    )
                    xt = xin.tile([128, S], bf16, tag=f"x{id(x_t)}_{k}", name=f"x{k}")
                    nc.sync.dma_start(
                        out=xt, in_=xdram[128 * k : 128 * k + 128, :]
                    )
                    x_t.append(xt)
            for k in range(HD // 128):
                nc.sync.dma_start(out=wo_t[k], in_=woT[128 * k : 128 * k + 128, :])

            # ---- projection-group emitters ----
            def proj_group(w_t, x_t, dst, n, m):
                ps = mps.tile([128, QCHUNK], fp32, tag="misc", name=f"pj{n}_{m}")
                for k in range(DT):
                    nc.tensor.matmul(
                        ps[:, :],
                        lhsT=w_t[k][:, 128 * m : 128 * m + 128],
                        rhs=x_t[k][:, QCHUNK * n : QCHUNK * (n + 1)],
                        start=(k == 0),
                        stop=(k == DT - 1),
                    )
                if (n + m) % 2 == 0:
                    nc.scalar.copy(
                        out=dst[:, m, QCHUNK * n : QCHUNK * (n + 1)],
                        in_=ps[:, :],
                    )
                else:
                    nc.vector.tensor_copy(
                        dst[:, m, QCHUNK * n : QCHUNK * (n + 1)], ps[:, :]
                    )

            def v_group(j):
                ps = mps.tile([128, HD], fp32, tag="misc", name=f"vj{j}")
                for k in range(DT):
                    nc.tensor.matmul(
                        ps[:, :],
                        lhsT=xv_t[k][:, 128 * j : 128 * j + 128],
                        rhs=wv_t[k][:, :],
                        start=(k == 0),
                        stop=(k == DT - 1),
                    )
                if j % 2 == 0:
                    nc.scalar.copy(
                        out=v_sb[:, j, :, 0:DK],
                        in_=ps.rearrange("p (h d) -> p h d", h=HPC),
                    )
                else:
                    nc.vector.tensor_copy(
                        v_sb[:, j, :, 0:DK],
                        ps.rearrange("p (h d) -> p h d", h=HPC),
                    )

            # chunk 0 prerequisites up front
            for m in range(2):
                proj_group(wq_t, xq_t, qt, 0, m)
            for m in range(2):
                proj_group(wk_t, xk_t, kt, 0, m)
            for j in range(4):
                v_group(j)

            # later QT/KT chunks, V tiles, and output projections stream into
            # the attention loop slots: queues[c] pops during chunk c.
            from collections import deque

            queues = [deque() for _ in range(NCHUNKS)]
            for n in range(1, NCHUNKS):
                for m in range(2):
                    queues[n - 1].append(
                        lambda n=n, m=m: proj_group(wq_t, xq_t, qt, n, m)
                    )
                for m in range(2):
                    queues[n - 1].append(
                        lambda n=n, m=m: proj_group(wk_t, xk_t, kt, n, m)
                    )
                for j in range(4 * n, 4 * n + 4):
                    queues[n - 1].append(lambda j=j: v_group(j))

            # ---- fused attention + streamed projections + output proj ----
            for c in range(NCHUNKS):
                qs = slice(QCHUNK * c, QCHUNK * (c + 1))
                nkt_c = 4 * (c + 1) if causal else NKT
                nsup = (nkt_c + KSUPER - 1) // KSUPER
                nslots = 2 * nsup
                pending = queues[c]
                per_slot = (len(pending) + nslots - 1) // max(1, nslots)

                osbs = {}
                for pair in range(2):
                    hs = (2 * pair, 2 * pair + 1)
                    pvs = {
                        h: pvps.tile(
                            [DK + 1, QCHUNK], fp32, tag="pv", name=f"pv{c}_{h}"
                        )
                        for h in hs
                    }
                    for s_i in range(nsup):
                        jt = [
                            KSUPER * s_i + j2
                            for j2 in range(KSUPER)
                            if KSUPER * s_i + j2 < nkt_c
                        ]
                        scs = {
                            h: scps.tile(
                                [128, KSUPER * QCHUNK], fp32, tag="sc",
                                name=f"sc{c}_{s_i}_{h}",
                            )
                            for h in hs
                        }
                        # adjacent QKs alternate PE row groups (rows 0-63 for
                        # even heads, 64-127 for odd) -> run concurrently
                        for j2, j in enumerate(jt):
                            for h in hs:
                                mh = h // 2
                                ph = 64 * (h % 2)
                                nc.tensor.matmul(
                                    scs[h][:, QCHUNK * j2 : QCHUNK * (j2 + 1)],
                                    lhsT=kt[ph : ph + DK, mh, 128 * j : 128 * j + 128],
                                    rhs=qt[ph : ph + DK, mh, qs],
                                    start=True,
                                    stop=True,
                                )
                        for h in hs:
                            pt = ptp.tile(
                                [128, KSUPER * QCHUNK], bf16, tag="pt",
                                name=f"pt{c}_{s_i}_{h}",
                            )
                            nsc = QCHUNK * len(jt)
                            nc.scalar.activation(
                                out=pt[:, 0:nsc],
                                in_=scs[h][:, 0:nsc],
                                func=mybir.ActivationFunctionType.Exp,
                                scale=1.0 / np.sqrt(np.float32(DK)),
                            )
                            for j2, j in enumerate(jt):
                                pslice = pt[:, QCHUNK * j2 : QCHUNK * (j2 + 1)]
                                if causal:
                                    off = 128 * j - QCHUNK * c
                                    if off >= 0:
                                        nc.vector.tensor_mul(
                                            out=pslice, in0=pslice, in1=emasks[off]
                                        )
                                else:
                                    mt = mload.tile(
                                        [128, QCHUNK], bf16, tag="mt",
                                        name=f"mt{c}_{s_i}_{h}_{j2}",
                                    )
                                    nc.sync.dma_start(
                                        out=mt,
                                        in_=maskT[128 * j : 128 * j + 128, qs],
                                    )
                                    nc.vector.tensor_mul(
                                        out=pslice, in0=pslice, in1=mt
                                    )
                                nc.tensor.matmul(
                                    pvs[h][0 : DK + 1, :],
                                    lhsT=v_sb[:, j, h, :],
                                    rhs=pslice,
                                    start=(j == 0),
                                    stop=(j == nkt_c - 1),
                                )
                        emitted = 0
                        while pending and emitted < per_slot:
                            pending.popleft()()
                            emitted += 1
                    # drain this pair's PV banks to SBUF
                    for h in hs:
                        if not causal:
                            nc.scalar.add(
                                out=pvs[h][DK : DK + 1, :],
                                in_=pvs[h][DK : DK + 1, :],
                                add=1e-30,
                            )
                        o = srec.tile(
                            [DK + 1, QCHUNK], fp32, tag="osb", bufs=6,
                            name=f"osb{c}_{h}",
                        )
                        nc.vector.tensor_copy(o[:, :], pvs[h][0 : DK + 1, :])
                        osbs[h] = o

                # normalize all 4 heads of this chunk (off critical path)
                recf = srec.tile([128, QCHUNK], fp32, tag="recf", name=f"recf{c}")
                recb = srec.tile([128, QCHUNK], bf16, tag="recb", name=f"recb{c}")
                for h in range(HPC):
                    nc.vector.tensor_copy(
                        recf[32 * h : 32 * h + 1, :],
                        osbs[h][DK : DK + 1, :],
                    )
                with nc.allow_low_precision(reason="softmax denom in bf16"):
                    nc.vector.reciprocal(out=recb[0:97, :], in_=recf[0:97, :])
                for h in range(HPC):
                    mh = h // 2
                    ph = 64 * (h % 2)
                    bc = mps.tile([DK, QCHUNK], fp32, tag="misc", name=f"bc{c}_{h}")
                    nc.tensor.matmul(
                        bc[:, :],
                        lhsT=ones[32 * h : 32 * h + 1, :],
                        rhs=recb[32 * h : 32 * h + 1, :],
                        start=True,
                        stop=True,
                        tile_position=(32 * h, 0),
                    )
                    nc.vector.tensor_mul(
                        out=at[ph : ph + DK, mh, qs],
                        in0=osbs[h][0:DK, :],
                        in1=bc[:, :],
                    )

                # output projection for this chunk streams into the next
                # chunk's slots (chunk 3's runs right here).
                def op_group(c, mo, qs=qs):
                    ps = mps.tile(
                        [128, QCHUNK], fp32, tag="misc", name=f"op{c}_{mo}"
                    )
                    for k2 in range(HD // 128):
                        nc.tensor.matmul(
                            ps[:, :],
                            lhsT=wo_t[k2][:, 128 * mo : 128 * mo + 128],
                            rhs=at[:, k2, qs],
                            start=(k2 == 0),
                            stop=(k2 == HD // 128 - 1),
                        )
                    stg = ostg.tile(
                        [128, QCHUNK], fp32, tag="stg", name=f"stg{c}_{mo}"
                    )
                    if mo % 2 == 0:
                        nc.scalar.copy(out=stg[:, :], in_=ps[:, :])
                    else:
                        nc.vector.tensor_copy(stg[:, :], ps[:, :])
                    nc.sync.dma_start(
                        out=yT[128 * mo : 128 * mo + 128, qs],
                        in_=stg[:, :],
                    )

                for mo in range(D // 128):
                    if c + 1 < NCHUNKS:
                        queues[c + 1].append(lambda c=c, mo=mo: op_group(c, mo))
                    else:
                        op_group(c, mo)

    _split_excess_waits(nc)
    return nc


def kernel(query, key, value, mask, W_q, W_k, W_v, W_o):
    from concourse.bass_utils import run_bass_kernel_spmd

    query = np.asarray(query)
    key = np.asarray(key)
    value = np.asarray(value)
    mask = np.asarray(mask)
    W_q = np.asarray(W_q)
    W_k = np.asarray(W_k)
    W_v = np.asarray(W_v)
    W_o = np.asarray(W_o)

    m2 = mask.reshape(mask.shape[-2], mask.shape[-1])
    causal = bool(
        np.array_equal(m2 != 0, np.tril(np.ones((S, S), dtype=bool)))
    )

    if causal not in _prog_cache:
        _prog_cache[causal] = _build_program(causal)
    nc = _prog_cache[causal]

    # host-side shard prep (bf16, transposed)
    xT = {}
    for b in range(B):
        xT[("q", b)] = np.ascontiguousarray(query[b].T).astype(BF16)
        xT[("k", b)] = np.ascontiguousarray(key[b].T).astype(BF16)
        xT[("v", b)] = np.ascontiguousarray(value[b].T).astype(BF16)
    maskT_np = None
    if not causal:
        maskT_np = np.ascontiguousarray((m2 != 0).T).astype(BF16)

    in_maps = []
    for c in range(NCORES):
        b, g = divmod(c, GROUPS)
        rows = slice(HD * g, HD * (g + 1))
        im = {
            "xqT": xT[("q", b)],
            "xkT": xT[("k", b)],
            "xvT": xT[("v", b)],
            "wqT": np.ascontiguousarray(W_q[rows, :].T).astype(BF16),
            "wkT": np.ascontiguousarray(W_k[rows, :].T).astype(BF16),
            "wvT": np.ascontiguousarray(W_v[rows, :].T).astype(BF16),
            "woT": np.ascontiguousarray(W_o[:, rows].T).astype(BF16),
        }
        if not causal:
            im["maskT"] = maskT_np
        in_maps.append(im)

    res = run_bass_kernel_spmd(nc, in_maps, core_ids=list(range(NCORES)))

    out = np.zeros((B, S, D), dtype=np.float32)
    for c in range(NCORES):
        b = c // GROUPS
        out[b] += res.results[c]["yT"].T
    return out


# revision 23
# speedup vs baseline: 1.4236x; 1.0056x over previous
"""Multi-head attention (B=2, S=2048, D=1024, H=16, causal) on 8 TRN2 cores.

Sharding: data-parallel over batch x tensor-parallel over heads (Megatron).
Core c handles batch b=c//4 and heads [4g, 4g+4) with g=c%4. Each core
computes its 4 heads' Q/K/V projections, causal attention, and its partial
output projection y_partial = attn_x @ W_o[:, cols].T; the host sums the 4
partials per batch.

Everything on-chip runs in transposed (feature x seq) layout so no
transposes are needed anywhere:
  QT/KT [256, 2048] = W @ x^T,  V [s, 4*65] with a fused ones-column,
  S^T[k, q] = KT_h.T @ QT_h,    P^T = exp(S^T/8) (ACT, scale folded),
  O^T_aug [65, q] = V_aug.T @ P^T  (row 64 = softmax denominator),
  attn^T = O^T[0:64] * bcast(1/denom),  y^T = WoT.T @ attn^T.
"""

import numpy as np
import ml_dtypes

B, S, D, H = 2, 2048, 1024, 16
DK = D // H  # 64
NCORES = 8
GROUPS = 4  # cores per batch
HPC = H // GROUPS  # heads per core = 4
HD = HPC * DK  # head dims per core = 256

BF16 = ml_dtypes.bfloat16

QCHUNK = 512  # q columns processed per softmax block
NCHUNKS = S // QCHUNK  # 4
KTILE = 128  # keys per matmul tile
NKT = S // KTILE  # 16
KSUPER = 2  # k-tiles per exp batch ([128, 1024] activations)

_prog_cache = {}


# --------------------------------------------------------------------------
# walrus workaround: this compiler build allows at most 1 semaphore wait per
# instruction; move excess waits onto NoOps inserted before the instruction.
def _split_excess_waits(nc):
    import concourse.mybir as mybir

    ctr = 0
    for f in nc.m.functions:
        for bb in f.blocks:
            out = []
            changed = False
            for inst in bb.instructions:
                si = inst.sync_info
                if si is not None and si.on_wait and len(si.on_wait) > 1:
                    waits = list(si.on_wait)
                    excess, keep = waits[:-1], waits[-1:]
                    for w in excess:
                        nop = mybir.InstNoOp(name=f"waitsplit-{ctr}", ins=[], outs=[])
                        ctr += 1
                        nop.engine = inst.engine
                        nop.sync_info = mybir.SyncInfo(on_wait=[w], on_update=[])
                        out.append(nop)
                    si.on_wait = keep
                    changed = True
                out.append(inst)
            if changed:
                bb.instructions = out
    return ctr


def _build_program(causal: bool):
    import concourse.bass as bass
    import concourse.mybir as mybir
    import concourse.tile as tile

    fp32 = mybir.dt.float32
    bf16 = mybir.dt.bfloat16

    nc = bass.Bass()

    xqT = nc.dram_tensor("xqT", [D, S], bf16, kind="ExternalInput")
    xkT = nc.dram_tensor("xkT", [D, S], bf16, kind="ExternalInput")
    xvT = nc.dram_tensor("xvT", [D, S], bf16, kind="ExternalInput")
    wqT = nc.dram_tensor("wqT", [D, HD], bf16, kind="ExternalInput")
    wkT = nc.dram_tensor("wkT", [D, HD], bf16, kind="ExternalInput")
    wvT = nc.dram_tensor("wvT", [D, HD], bf16, kind="ExternalInput")
    woT = nc.dram_tensor("woT", [HD, D], bf16, kind="ExternalInput")
    yT = nc.dram_tensor("yT", [D, S], mybir.dt.float32, kind="ExternalOutput")
    maskT = None
    if not causal:
        maskT = nc.dram_tensor("maskT", [S, S], bf16, kind="ExternalInput")

    DT = D // 128  # 8 contraction tiles for the input projections

    with tile.TileContext(nc) as tc:
        with (
            tc.tile_pool(name="wpool", bufs=1) as wpool,
            tc.tile_pool(name="res", bufs=1) as res,
            tc.tile_pool(name="xin", bufs=1) as xin,
            tc.tile_pool(name="small", bufs=1) as small,
            tc.tile_pool(name="scps", bufs=2, space="PSUM") as scps,
            tc.tile_pool(name="pvps", bufs=2, space="PSUM") as pvps,
            tc.tile_pool(name="mps", bufs=2, space="PSUM") as mps,
            tc.tile_pool(name="pt", bufs=4) as ptp,
            tc.tile_pool(name="srec", bufs=2) as srec,
            tc.tile_pool(name="ostg", bufs=3) as ostg,
            tc.tile_pool(name="mload", bufs=4) as mload,
        ):
            wq_t = [wpool.tile([128, HD], bf16, tag=f"wq{k}", name=f"wq{k}") for k in range(DT)]
            wk_t = [wpool.tile([128, HD], bf16, tag=f"wk{k}", name=f"wk{k}") for k in range(DT)]
            wv_t = [wpool.tile([128, HD], bf16, tag=f"wv{k}", name=f"wv{k}") for k in range(DT)]
            wo_t = [wpool.tile([128, D], bf16, tag=f"wo{k}", name=f"wo{k}") for k in range(HD // 128)]

            # resident activations, all in (feature x seq) layout
            qt = res.tile([128, 2, S], bf16, tag="qt")
            kt = res.tile([128, 2, S], bf16, tag="kt")
            v_sb = res.tile([128, NKT, HPC, DK + 1], bf16, tag="v")
            at = res.tile([128, 2, S], bf16, tag="at")

            nc.vector.memset(v_sb[:, :, :, DK : DK + 1], 1.0)
            ones = small.tile([128, DK], bf16, tag="ones")
            nc.vector.memset(ones, 1.0)

            emasks = {}
            if causal:
                for off in (0, 128, 256, 384):
                    m = small.tile([128, QCHUNK], bf16, tag=f"emask{off}", name=f"emask{off}")
                    nc.gpsimd.memset(m, 1.0)
                    nc.gpsimd.affine_select(
                        out=m,
                        in_=m,
                        compare_op=mybir.AluOpType.is_ge,
                        fill=0.0,
                        base=-off,
                        pattern=[[1, QCHUNK]],
                        channel_multiplier=-1,
                    )
                    emasks[off] = m

            # ---- input DMAs (issue order = arrival order) ----
            xq_t, xk_t, xv_t = [], [], []
            for w_t, wdram, x_t, xdram in (
                (wq_t, wqT, xq_t, xqT),
                (wk_t, wkT, xk_t, xkT),
                (wv_t, wvT, xv_t, xvT),
            ):
                for k in range(DT):
                    nc.sync.dma_start(
                        out=w_t[k], in_=wdram[128 * k : 128 * k + 128, :]
                    )
                    xt = xin.tile([128, S], bf16, tag=f"x{id(x_t)}_{k}", name=f"x{k}")
                    nc.sync.dma_start(
                        out=xt, in_=xdram[128 * k : 128 * k + 128, :]
                    )
                    x_t.append(xt)
            for k in range(HD // 128):
                nc.sync.dma_start(out=wo_t[k], in_=woT[128 * k : 128 * k + 128, :])

            # ---- projection-group emitters ----
            def proj_group(w_t, x_t, dst, n, m):
                ps = mps.tile([128, QCHUNK], fp32, tag="misc", name=f"pj{n}_{m}")
                for k in range(DT):
                    nc.tensor.matmul(
                        ps[:, :],
                        lhsT=w_t[k][:, 128 * m : 128 * m + 128],
                        rhs=x_t[k][:, QCHUNK * n : QCHUNK * (n + 1)],
                        start=(k == 0),
                        stop=(k == DT - 1),
                    )
                if (n + m) % 2 == 0:
                    nc.scalar.copy(
                        out=dst[:, m, QCHUNK * n : QCHUNK * (n + 1)],
                        in_=ps[:, :],
                    )
                else:
                    nc.vector.tensor_copy(
                        dst[:, m, QCHUNK * n : QCHUNK * (n + 1)], ps[:, :]
                    )

            def v_group(j):
                ps = mps.tile([128, HD], fp32, tag="misc", name=f"vj{j}")
                for k in range(DT):
                    nc.tensor.matmul(
                        ps[:, :],
                        lhsT=xv_t[k][:, 128 * j : 128 * j + 128],
                        rhs=wv_t[k][:, :],
                        start=(k == 0),
                        stop=(k == DT - 1),
                    )
                if j % 2 == 0:
                    nc.scalar.copy(
                        out=v_sb[:, j, :, 0:DK],
                        in_=ps.rearrange("p (h d) -> p h d", h=HPC),
                    )
                else:
                    nc.vector.tensor_copy(
                        v_sb[:, j, :, 0:DK],
                        ps.rearrange("p (h d) -> p h d", h=HPC),
                    )

            # chunk 0 prerequisites up front
            for m in range(2):
                proj_group(wq_t, xq_t, qt, 0, m)
            for m in range(2):
                proj_group(wk_t, xk_t, kt, 0, m)

            # later QT/KT chunks, V tiles, and output projections stream into
            # the attention loop slots: queues[c] pops during chunk c.
            from collections import deque

            queues = [deque() for _ in range(NCHUNKS)]
            for j in range(4):
                queues[0].append(lambda j=j: v_group(j))
            for n in range(1, NCHUNKS):
                for m in range(2):
                    queues[n - 1].append(
                        lambda n=n, m=m: proj_group(wq_t, xq_t, qt, n, m)
                    )
                for m in range(2):
                    queues[n - 1].append(
                        lambda n=n, m=m: proj_group(wk_t, xk_t, kt, n, m)
                    )
                for j in range(4 * n, 4 * n + 4):
                    queues[n - 1].append(lambda j=j: v_group(j))

            # ---- fused attention + streamed projections + output proj ----
            for c in range(NCHUNKS):
                qs = slice(QCHUNK * c, QCHUNK * (c + 1))
                nkt_c = 4 * (c + 1) if causal else NKT
                nsup = (nkt_c + KSUPER - 1) // KSUPER
                nslots = 2 * nsup
                pending = queues[c]
                per_slot = (len(pending) + nslots - 1) // max(1, nslots)

                osbs = {}
                for pair in range(2):
                    hs = (2 * pair, 2 * pair + 1)
                    pvs = {
                        h: pvps.tile(
                            [DK + 1, QCHUNK], fp32, tag="pv", name=f"pv{c}_{h}"
                        )
                        for h in hs
                    }
                    for s_i in range(nsup):
                        jt = [
                            KSUPER * s_i + j2
                            for j2 in range(KSUPER)
                            if KSUPER * s_i + j2 < nkt_c
                        ]
                        scs = {
                            h: scps.tile(
                                [128, KSUPER * QCHUNK], fp32, tag="sc",
                                name=f"sc{c}_{s_i}_{h}",
                            )
                            for h in hs
                        }
                        # adjacent QKs alternate PE row groups (rows 0-63 for
                        # even heads, 64-127 for odd) -> run concurrently
                        for j2, j in enumerate(jt):
                            for h in hs:
                                mh = h // 2
                                ph = 64 * (h % 2)
                                nc.tensor.matmul(
                                    scs[h][:, QCHUNK * j2 : QCHUNK * (j2 + 1)],
                                    lhsT=kt[ph : ph + DK, mh, 128 * j : 128 * j + 128],
                                    rhs=qt[ph : ph + DK, mh, qs],
                                    start=True,
                                    stop=True,
                                )
                        pts = {}
                        for h in hs:
                            pt = ptp.tile(
                                [128, KSUPER * QCHUNK], bf16, tag="pt",
                                name=f"pt{c}_{s_i}_{h}",
                            )
                            pts[h] = pt
                            nsc = QCHUNK * len(jt)
                            nc.scalar.activation(
                                out=pt[:, 0:nsc],
                                in_=scs[h][:, 0:nsc],
                                func=mybir.ActivationFunctionType.Exp,
                                scale=1.0 / np.sqrt(np.float32(DK)),
                            )
                        emitted = 0
                        while pending and emitted < per_slot:
                            pending.popleft()()
                            emitted += 1
                        for h in hs:
                            pt = pts[h]
                            for j2, j in enumerate(jt):
                                pslice = pt[:, QCHUNK * j2 : QCHUNK * (j2 + 1)]
                                if causal:
                                    off = 128 * j - QCHUNK * c
                                    if off >= 0:
                                        nc.vector.tensor_mul(
                                            out=pslice, in0=pslice, in1=emasks[off]
                                        )
                                else:
                                    mt = mload.tile(
                                        [128, QCHUNK], bf16, tag="mt",
                                        name=f"mt{c}_{s_i}_{h}_{j2}",
                                    )
                                    nc.sync.dma_start(
                                        out=mt,
                                        in_=maskT[128 * j : 128 * j + 128, qs],
                                    )
                                    nc.vector.tensor_mul(
                                        out=pslice, in0=pslice, in1=mt
                                    )
                                nc.tensor.matmul(
                                    pvs[h][0 : DK + 1, :],
                                    lhsT=v_sb[:, j, h, :],
                                    rhs=pslice,
                                    start=(j == 0),
                                    stop=(j == nkt_c - 1),
                                )
                    # drain this pair's PV banks to SBUF
                    for h in hs:
                        if not causal:
                            nc.scalar.add(
                                out=pvs[h][DK : DK + 1, :],
                                in_=pvs[h][DK : DK + 1, :],
                                add=1e-30,
                            )
                        o = srec.tile(
                            [DK + 1, QCHUNK], fp32, tag="osb", bufs=6,
                            name=f"osb{c}_{h}",
                        )
                        nc.vector.tensor_copy(o[:, :], pvs[h][0 : DK + 1, :])
                        osbs[h] = o

                # normalize all 4 heads of this chunk (off critical path)
                recf = srec.tile([128, QCHUNK], fp32, tag="recf", name=f"recf{c}")
                recb = srec.tile([128, QCHUNK], bf16, tag="recb", name=f"recb{c}")
                for h in range(HPC):
                    nc.vector.tensor_copy(
                        recf[32 * h : 32 * h + 1, :],
                        osbs[h][DK : DK + 1, :],
                    )
                with nc.allow_low_precision(reason="softmax denom in bf16"):
                    nc.vector.reciprocal(out=recb[0:97, :], in_=recf[0:97, :])
                for h in range(HPC):
                    mh = h // 2
                    ph = 64 * (h % 2)
                    bc = mps.tile([DK, QCHUNK], fp32, tag="misc", name=f"bc{c}_{h}")
                    nc.tensor.matmul(
                        bc[:, :],
                        lhsT=ones[32 * h : 32 * h + 1, :],
                        rhs=recb[32 * h : 32 * h + 1, :],
                        start=True,
                        stop=True,
                        tile_position=(32 * h, 0),
                    )
                    nc.vector.tensor_mul(
                        out=at[ph : ph + DK, mh, qs],
                        in0=osbs[h][0:DK, :],
                        in1=bc[:, :],
                    )

                # output projection for this chunk streams into the next
                # chunk's slots (chunk 3's runs right here).
                def op_group(c, mo, qs=qs):
                    ps = mps.tile(
                        [128, QCHUNK], fp32, tag="misc", name=f"op{c}_{mo}"
                    )
                    for k2 in range(HD // 128):
                        nc.tensor.matmul(
                            ps[:, :],
                            lhsT=wo_t[k2][:, 128 * mo : 128 * mo + 128],
                            rhs=at[:, k2, qs],
                            start=(k2 == 0),
                            stop=(k2 == HD // 128 - 1),
                        )
                    stg = ostg.tile(
                        [128, QCHUNK], fp32, tag="stg", name=f"stg{c}_{mo}"
                    )
                    if mo % 2 == 0:
                        nc.scalar.copy(out=stg[:, :], in_=ps[:, :])
                    else:
                        nc.vector.tensor_copy(stg[:, :], ps[:, :])
                    nc.sync.dma_start(
                        out=yT[128 * mo : 128 * mo + 128, qs],
                        in_=stg[:, :],
                    )

                for mo in range(D // 128):
                    if c + 1 < NCHUNKS:
                        queues[c + 1].append(lambda c=c, mo=mo: op_group(c, mo))
                    else:
                        op_group(c, mo)

    _split_excess_waits(nc)
    return nc


def kernel(query, key, value, mask, W_q, W_k, W_v, W_o):
    from concourse.bass_utils import run_bass_kernel_spmd

    query = np.asarray(query)
    key = np.asarray(key)
    value = np.asarray(value)
    mask = np.asarray(mask)
    W_q = np.asarray(W_q)
    W_k = np.asarray(W_k)
    W_v = np.asarray(W_v)
    W_o = np.asarray(W_o)

    m2 = mask.reshape(mask.shape[-2], mask.shape[-1])
    causal = bool(
        np.array_equal(m2 != 0, np.tril(np.ones((S, S), dtype=bool)))
    )

    if causal not in _prog_cache:
        _prog_cache[causal] = _build_program(causal)
    nc = _prog_cache[causal]

    # host-side shard prep (bf16, transposed)
    xT = {}
    for b in range(B):
        xT[("q", b)] = np.ascontiguousarray(query[b].T).astype(BF16)
        xT[("k", b)] = np.ascontiguousarray(key[b].T).astype(BF16)
        xT[("v", b)] = np.ascontiguousarray(value[b].T).astype(BF16)
    maskT_np = None
    if not causal:
        maskT_np = np.ascontiguousarray((m2 != 0).T).astype(BF16)

    in_maps = []
    for c in range(NCORES):
        b, g = divmod(c, GROUPS)
        rows = slice(HD * g, HD * (g + 1))
        im = {
            "xqT": xT[("q", b)],
            "xkT": xT[("k", b)],
            "xvT": xT[("v", b)],
            "wqT": np.ascontiguousarray(W_q[rows, :].T).astype(BF16),
            "wkT": np.ascontiguousarray(W_k[rows, :].T).astype(BF16),
            "wvT": np.ascontiguousarray(W_v[rows, :].T).astype(BF16),
            "woT": np.ascontiguousarray(W_o[:, rows].T).astype(BF16),
        }
        if not causal:
            im["maskT"] = maskT_np
        in_maps.append(im)

    res = run_bass_kernel_spmd(nc, in_maps, core_ids=list(range(NCORES)))

    out = np.zeros((B, S, D), dtype=np.float32)
    for c in range(NCORES):
        b = c // GROUPS
        out[b] += res.results[c]["yT"].T
    return out
